# revision 1
# baseline (speedup 1.0000x reference)
"""GCN+GIN graph encoder on 8 Trainium2 NeuronCores (Bass/Tile).

Math (reference):
  GCNConv:  h = relu(segsum_dst(norm_e * (x@W0)[src]) + b0),
            norm_e = dinv[src]*dinv[dst] over edges+self-loops,
            dinv = rsqrt(deg incl self-loop)
  GIN x2:   h = relu((h + segsum_dst(h[src])) @ Wg + bg)
  pool:     m = segment_mean(h, batch) -> relu(m@Wh1+bh1)@Wh2+bh2

Distribution: nodes (and their in-edges) sharded contiguously over 8 cores.
Per layer each core aggregates messages for its own dst nodes by gathering
rows of a replicated node-feature table (dma_gather, 1024-row packed ops on
4 SWDGE queues), reducing edge tiles with one-hot selection matrices on the
TensorEngine, applying the layer linear transform W-stationary in feat-major,
then transposing back to node-major.  Tables are re-replicated between layers
with an AllGather; pooled partial means are combined with an AllReduce and
the small MLP head is computed redundantly on every core.

Aggregation identity per dst block b (128 dst nodes):
  aggT[f, d] = sum_e msg[e, f] * sel[e, d],  sel[e, d] = (doff[e] == d) * val[e]
computed as matmul(lhsT=msg_tile[128e, 128f], rhs=sel[128e, 128d]) accumulated
in PSUM over the block's edge tiles.  GCN folds dinv[src] into the table rows
(host-prescaled x) and dinv[dst] into val; GIN uses val=1 and a self-loop edge
supplies the "+h" term.  Pad edge slots carry doff=-1 -> zero contribution.
"""
import sys
import os

sys.path.insert(0, '/opt/trn_rl_repo')

import numpy as np

import concourse.bass as bass
import concourse.bacc as bacc
import concourse.mybir as mybir
import concourse.tile as tile
from concourse.bass_utils import run_bass_kernel_spmd
from concourse.masks import make_identity

F32 = mybir.dt.float32
I16 = mybir.dt.int16
P = 128
NCORES = 8
GATHER_ROWS = 1024          # rows per dma_gather (single_packet limit)
NQ = 4                      # SWDGE queues


class Cfg:
    def __init__(self, N, E, G, F, NHID, NOUT, NPN):
        self.N = N            # real nodes
        self.E = E            # edges (no self loops)
        self.G = G            # graphs
        self.F = F            # feature/hidden width (128)
        self.NHID = NHID
        self.NOUT = NOUT
        self.NPN = NPN        # real nodes per core
        assert NPN * NCORES >= N > NPN * (NCORES - 1)
        self.NPC = ((NPN + P - 1) // P) * P   # padded nodes per core
        self.NBLK = self.NPC // P
        self.NPAD = self.NPC * NCORES
        self.NHALF = self.NPAD // 2
        assert self.NHALF < 32768
        assert G == 2 * P


FULL = Cfg(N=50000, E=800000, G=256, F=128, NHID=256, NOUT=128, NPN=6250)


# ---------------------------------------------------------------- host prep
def preprocess(cfg, x, edge_index, batch, W0, b0, Wg1, bg1, Wg2, bg2,
               Wh1, bh1, Wh2, bh2):
    N, G, F = cfg.N, cfg.G, cfg.F
    NPN, NPC, NBLK, NPAD, NHALF = cfg.NPN, cfg.NPC, cfg.NBLK, cfg.NPAD, cfg.NHALF

    src = np.asarray(edge_index[0], dtype=np.int64)
    dst = np.asarray(edge_index[1], dtype=np.int64)
    batch = np.asarray(batch, dtype=np.int64)
    loop = np.arange(N, dtype=np.int64)
    s_all = np.concatenate([src, loop])
    d_all = np.concatenate([dst, loop])

    deg = np.bincount(d_all, minlength=N).astype(np.float64)
    dinv = (1.0 / np.sqrt(np.maximum(deg, 1.0))).astype(np.float32)

    def tabidx(n):
        c = n // NPN
        return c * NPC + (n - c * NPN)

    sidx = tabidx(s_all).astype(np.int64)
    c_e = d_all // NPN
    loc = d_all - c_e * NPN
    b_e = loc // P
    off_e = loc % P
    gblk = c_e * NBLK + b_e                      # global dst block id
    val_e = dinv[d_all].astype(np.float32)      # GCN dst scaling

    NGB = NCORES * NBLK
    streams = {}
    for name, mask in (("lo", sidx < NHALF), ("hi", sidx >= NHALF)):
        sg = gblk[mask]
        si = sidx[mask] - (0 if name == "lo" else NHALF)
        sof = off_e[mask]
        sva = val_e[mask]
        order = np.argsort(sg, kind="stable")
        sg, si, sof, sva = sg[order], si[order], sof[order], sva[order]
        cnt = np.bincount(sg, minlength=NGB)
        # per-BLOCK-INDEX tile counts: max over the 8 cores only (SPMD allows
        # per-block variation, just not per-core) -> much less padding than a
        # global max over all core*block pairs
        NTb = np.ceil(cnt.reshape(NCORES, NBLK).max(axis=0) / P).astype(np.int64)
        rows_blk_b = NTb * P                      # [NBLK]
        blk_starts = np.zeros(NBLK, dtype=np.int64)
        blk_starts[1:] = np.cumsum(rows_blk_b)[:-1]
        rows_core = int(rows_blk_b.sum())
        starts = np.zeros(NGB, dtype=np.int64)
        starts[1:] = np.cumsum(cnt)[:-1]
        rank = np.arange(len(sg)) - np.repeat(starts, cnt)
        c_of = sg // NBLK
        b_of = sg % NBLK
        pos = c_of * rows_core + blk_starts[b_of] + rank
        tot = NCORES * rows_core
        idx_arr = np.zeros(tot, dtype=np.int32)
        doff_arr = np.full(tot, -1.0, dtype=np.float32)
        val_arr = np.zeros(tot, dtype=np.float32)
        idx_arr[pos] = si
        doff_arr[pos] = sof
        val_arr[pos] = sva
        idx_arr = idx_arr.reshape(NCORES, rows_core)
        doff_arr = doff_arr.reshape(NCORES, rows_core)
        val_arr = val_arr.reshape(NCORES, rows_core)
        NG = (rows_core + GATHER_ROWS - 1) // GATHER_ROWS
        rows_g = NG * GATHER_ROWS
        pad = rows_g - rows_core
        if pad:
            idx_arr = np.pad(idx_arr, ((0, 0), (0, pad)))
        # wrap int16 for dma_gather: element i -> partition i%16, col i//16
        NWG = GATHER_ROWS // 16
        wrapped = idx_arr.reshape(NCORES, NG, NWG, 16).transpose(0, 3, 1, 2)
        wrapped = wrapped.reshape(NCORES, 16, NG * NWG).astype(np.int16)
        wrapped = np.tile(wrapped, (1, 8, 1))    # [NCORES, 128, NG*NWG]
        # doff/val tile-major: [T=sum(NTb), 128] -> [128, T]
        T = rows_core // P
        doff2 = doff_arr.reshape(NCORES, T, P).transpose(0, 2, 1).copy()
        val2 = val_arr.reshape(NCORES, T, P).transpose(0, 2, 1).copy()
        tile_base = (blk_starts // P).tolist()
        streams[name] = dict(NTb=NTb.tolist(), tile_base=tile_base, T=T, NG=NG,
                             idx=wrapped, doff=doff2, val=val2)

    # per-core node-feature slice, pre-scaled by dinv (GCN source scaling)
    xs = np.zeros((NCORES, NPC, F), dtype=np.float32)
    x = np.asarray(x, dtype=np.float32)
    for c in range(NCORES):
        lo_n = c * NPN
        hi_n = min(N, (c + 1) * NPN)
        n = hi_n - lo_n
        xs[c, :n] = x[lo_n:hi_n] * dinv[lo_n:hi_n, None]

    # pooling metadata
    cnt_g = np.bincount(batch, minlength=G).astype(np.float32)
    invc = (1.0 / np.maximum(cnt_g, 1.0)).astype(np.float32)
    batA = np.full((NCORES, P, NBLK), -1.0, dtype=np.float32)
    batB = np.full((NCORES, P, NBLK), -1000.0, dtype=np.float32)
    for c in range(NCORES):
        lo_n = c * NPN
        hi_n = min(N, (c + 1) * NPN)
        n = hi_n - lo_n
        bb = batch[lo_n:hi_n].astype(np.float32)
        colmaj = np.full(NPC, -1.0, dtype=np.float32)
        colmaj[:n] = bb
        batA[c] = colmaj.reshape(NBLK, P).T
        batB[c] = batA[c] - 128.0
        batA[c][batA[c] < 0] = -1.0

    iota = np.broadcast_to(np.arange(P, dtype=np.float32), (P, P)).copy()

    common = dict(
        iota=iota,
        w0=np.asarray(W0, np.float32), wg1=np.asarray(Wg1, np.float32),
        wg2=np.asarray(Wg2, np.float32),
        b0c=np.asarray(b0, np.float32).reshape(P, 1).copy(),
        bg1c=np.asarray(bg1, np.float32).reshape(P, 1).copy(),
        bg2c=np.asarray(bg2, np.float32).reshape(P, 1).copy(),
        wh1=np.asarray(Wh1, np.float32),
        bh1c=np.asarray(bh1, np.float32).reshape(2, P).T.copy(),  # [128,2]
        wh2=np.asarray(Wh2, np.float32),
        bh2rep=np.broadcast_to(np.asarray(bh2, np.float32), (P, cfg.NOUT)).copy(),
        invcA=invc[:P].reshape(P, 1).copy(),
        invcB=invc[P:].reshape(P, 1).copy(),
    )
    in_maps = []
    for c in range(NCORES):
        m = dict(common)
        m.update(
            xs=xs[c],
            idxlo=streams["lo"]["idx"][c], idxhi=streams["hi"]["idx"][c],
            dofflo=streams["lo"]["doff"][c], doffhi=streams["hi"]["doff"][c],
            vallo=streams["lo"]["val"][c], valhi=streams["hi"]["val"][c],
            batA=batA[c], batB=batB[c],
        )
        in_maps.append(m)
    meta = dict(NTBLO=streams["lo"]["NTb"], BASELO=streams["lo"]["tile_base"],
                TLO=streams["lo"]["T"], NGLO=streams["lo"]["NG"],
                NTBHI=streams["hi"]["NTb"], BASEHI=streams["hi"]["tile_base"],
                THI=streams["hi"]["T"], NGHI=streams["hi"]["NG"])
    return in_maps, meta


# ---------------------------------------------------------------- program
def build_program(cfg, meta):
    NPC, NBLK, NPAD, NHALF = cfg.NPC, cfg.NBLK, cfg.NPAD, cfg.NHALF
    F, NHID, NOUT, G = cfg.F, cfg.NHID, cfg.NOUT, cfg.G
    NTBLO, BASELO, TLO, NGLO = meta["NTBLO"], meta["BASELO"], meta["TLO"], meta["NGLO"]
    NTBHI, BASEHI, THI, NGHI = meta["NTBHI"], meta["BASEHI"], meta["THI"], meta["NGHI"]
    NWG = GATHER_ROWS // 16
    CHUNKS = GATHER_ROWS // P     # 8 message tiles per gather

    nc = bacc.Bacc(None, target_bir_lowering=False, debug=True,
                   num_devices=NCORES, num_swdge_queues=NQ)

    def din(name, shape, dt=F32):
        return nc.declare_dram_parameter(name, list(shape), dt, isOutput=False)

    xs_d = din("xs", [NPC, F])
    idxlo_d = din("idxlo", [P, NGLO * NWG], I16)
    idxhi_d = din("idxhi", [P, NGHI * NWG], I16)
    dofflo_d = din("dofflo", [P, TLO])
    doffhi_d = din("doffhi", [P, THI])
    vallo_d = din("vallo", [P, TLO])
    valhi_d = din("valhi", [P, THI])
    iota_d = din("iota", [P, P])
    w0_d = din("w0", [F, F]); wg1_d = din("wg1", [F, F]); wg2_d = din("wg2", [F, F])
    b0c_d = din("b0c", [P, 1]); bg1c_d = din("bg1c", [P, 1]); bg2c_d = din("bg2c", [P, 1])
    wh1_d = din("wh1", [F, NHID]); bh1c_d = din("bh1c", [P, 2])
    wh2_d = din("wh2", [NHID, NOUT]); bh2rep_d = din("bh2rep", [P, NOUT])
    batA_d = din("batA", [P, NBLK]); batB_d = din("batB", [P, NBLK])
    invcA_d = din("invcA", [P, 1]); invcB_d = din("invcB", [P, 1])
    out_d = nc.declare_dram_parameter("out", [G, NOUT], F32, isOutput=True)

    slice0 = nc.dram_tensor("slice0", [NPC, F], F32)
    slice1 = nc.dram_tensor("slice1", [NPC, F], F32)
    slice2 = nc.dram_tensor("slice2", [NPC, F], F32)
    tab1 = nc.dram_tensor("tab1", [NPAD, F], F32)
    tab2 = nc.dram_tensor("tab2", [NPAD, F], F32)
    tab3 = nc.dram_tensor("tab3", [NPAD, F], F32)
    pool_in = nc.dram_tensor("pool_in", [G, F], F32)
    pool_out = nc.dram_tensor("pool_out", [G, F], F32, addr_space="Shared")
    groups = [list(range(NCORES))]

    with tile.TileContext(nc) as tc:
        with (
            tc.tile_pool(name="const", bufs=1) as constp,
            tc.tile_pool(name="meta", bufs=1) as metap,
            tc.tile_pool(name="msg", bufs=6) as msgp,
            tc.tile_pool(name="sel", bufs=4) as selp,
            tc.tile_pool(name="work", bufs=6) as workp,
            tc.tile_pool(name="pagg", bufs=2, space="PSUM") as pagg,
            tc.tile_pool(name="phT", bufs=2, space="PSUM") as phT,
            tc.tile_pool(name="ptr", bufs=1, space="PSUM") as ptr,
            tc.tile_pool(name="ppool", bufs=1, space="PSUM") as ppool,
        ):
            # ---- constants / metadata to SBUF
            ident = constp.tile([P, P], F32)
            make_identity(nc, ident[:])
            iota = constp.tile([P, P], F32)
            nc.sync.dma_start(out=iota[:], in_=iota_d[:])

            def load(t_shape, dram, dt=F32, pool=metap):
                nm = f"sb_{dram.name}"
                t = pool.tile(list(t_shape), dt, name=nm, tag=nm)
                nc.sync.dma_start(out=t[:], in_=dram[:])
                return t

            idxlo = load([P, NGLO * NWG], idxlo_d, I16)
            idxhi = load([P, NGHI * NWG], idxhi_d, I16)
            dofflo = load([P, TLO], dofflo_d)
            doffhi = load([P, THI], doffhi_d)
            vallo = load([P, TLO], vallo_d)
            valhi = load([P, THI], valhi_d)
            w0 = load([F, F], w0_d, pool=constp)
            wg1 = load([F, F], wg1_d, pool=constp)
            wg2 = load([F, F], wg2_d, pool=constp)
            b0c = load([P, 1], b0c_d, pool=constp)
            bg1c = load([P, 1], bg1c_d, pool=constp)
            bg2c = load([P, 1], bg2c_d, pool=constp)
            wh1 = load([F, NHID], wh1_d, pool=constp)
            bh1c = load([P, 2], bh1c_d, pool=constp)
            wh2 = constp.tile([P, (NHID // P) * NOUT], F32)
            for h in range(NHID // P):
                nc.sync.dma_start(out=wh2[:, h * NOUT:(h + 1) * NOUT],
                                  in_=wh2_d[h * P:(h + 1) * P, :])
            bh2rep = load([P, NOUT], bh2rep_d, pool=constp)
            batA = load([P, NBLK], batA_d, pool=constp)
            batB = load([P, NBLK], batB_d, pool=constp)
            invcA = load([P, 1], invcA_d, pool=constp)
            invcB = load([P, 1], invcB_d, pool=constp)

            # stage xs -> slice0 -> tab1 (collectives need internal tensors)
            for b in range(NBLK):
                t = workp.tile([P, F], F32)
                nc.sync.dma_start(out=t[:], in_=xs_d[b * P:(b + 1) * P, :])
                nc.sync.dma_start(out=slice0[b * P:(b + 1) * P, :], in_=t[:])
            nc.gpsimd.collective_compute(
                "AllGather", mybir.AluOpType.bypass, replica_groups=groups,
                ins=[slice0[:]], outs=[tab1[:]])

            pool_ps = {}

            def emit_layer(L, tab, W_sb, bias_col, use_val, out_slice):
                stream_info = [
                    ("lo", NTBLO, BASELO, idxlo, dofflo, vallo, tab[0:NHALF, :]),
                    ("hi", NTBHI, BASEHI, idxhi, doffhi, valhi, tab[NHALF:NPAD, :]),
                ]
                gbufs = {"lo": {}, "hi": {}}

                def get_gather(sname, g, idx_sb, tab_ap):
                    d = gbufs[sname]
                    if g not in d:
                        buf = msgp.tile([P, GATHER_ROWS], F32)
                        nc.gpsimd.dma_gather(
                            out_ap=buf[:].rearrange("p (c f) -> p c f", f=F),
                            in_ap=tab_ap,
                            idxs_ap=idx_sb[:, g * NWG:(g + 1) * NWG],
                            num_idxs=GATHER_ROWS, num_idxs_reg=GATHER_ROWS,
                            elem_size=F, single_packet=True,
                            queue_num=(L * NBLK + g) % NQ)
                        d[g] = buf
                    return d[g]

                for b in range(NBLK):
                    agg_ps = pagg.tile([P, F], F32, space="PSUM", tag="agg")
                    work = []
                    for sname, NTB, BASE, idx_sb, doff_sb, val_sb, tab_ap in stream_info:
                        for tt in range(NTB[b]):
                            work.append((sname, BASE[b] + tt, idx_sb, doff_sb,
                                         val_sb, tab_ap))
                    for wi, (sname, t, idx_sb, doff_sb, val_sb, tab_ap) in enumerate(work):
                        g, ch = divmod(t, CHUNKS)
                        buf = get_gather(sname, g, idx_sb, tab_ap)
                        sel = selp.tile([P, P], F32)
                        col = slice(t, t + 1)
                        if use_val:
                            nc.vector.tensor_scalar(
                                out=sel[:], in0=iota[:],
                                scalar1=doff_sb[:, col],
                                scalar2=val_sb[:, col],
                                op0=mybir.AluOpType.is_equal,
                                op1=mybir.AluOpType.mult)
                        else:
                            nc.vector.tensor_scalar(
                                out=sel[:], in0=iota[:],
                                scalar1=doff_sb[:, col], scalar2=None,
                                op0=mybir.AluOpType.is_equal)
                        nc.tensor.matmul(
                            out=agg_ps[:],
                            lhsT=buf[:, ch * F:(ch + 1) * F],
                            rhs=sel[:], start=(wi == 0),
                            stop=(wi == len(work) - 1))
                    aggT = workp.tile([P, F], F32)
                    nc.vector.tensor_copy(out=aggT[:], in_=agg_ps[:])
                    hT_ps = phT.tile([P, F], F32, space="PSUM", tag="hT")
                    nc.tensor.matmul(out=hT_ps[:], lhsT=W_sb[:], rhs=aggT[:],
                                     start=True, stop=True)
                    hT = workp.tile([P, F], F32)
                    nc.scalar.activation(out=hT[:], in_=hT_ps[:],
                                         func=mybir.ActivationFunctionType.Relu,
                                         bias=bias_col[:, 0:1])
                    h_ps = ptr.tile([P, F], F32, space="PSUM", tag="tr")
                    nc.tensor.transpose(out=h_ps[:], in_=hT[:], identity=ident[:])
                    h_sb = workp.tile([P, F], F32)
                    nc.vector.tensor_copy(out=h_sb[:], in_=h_ps[:])
                    if out_slice is not None:
                        nc.sync.dma_start(out=out_slice[b * P:(b + 1) * P, :],
                                          in_=h_sb[:])
                    else:
                        for half, bat in (("A", batA), ("B", batB)):
                            if half not in pool_ps:
                                pool_ps[half] = ppool.tile(
                                    [P, F], F32, space="PSUM",
                                    tag=f"pool{half}", name=f"pool{half}")
                            selp_t = selp.tile([P, P], F32)
                            nc.vector.tensor_scalar(
                                out=selp_t[:], in0=iota[:],
                                scalar1=bat[:, b:b + 1], scalar2=None,
                                op0=mybir.AluOpType.is_equal)
                            nc.tensor.matmul(
                                out=pool_ps[half][:], lhsT=selp_t[:], rhs=h_sb[:],
                                start=(b == 0), stop=(b == NBLK - 1))

            emit_layer(0, tab1, w0, b0c, True, slice1)
            nc.gpsimd.collective_compute(
                "AllGather", mybir.AluOpType.bypass, replica_groups=groups,
                ins=[slice1[:]], outs=[tab2[:]])
            emit_layer(1, tab2, wg1, bg1c, False, slice2)
            nc.gpsimd.collective_compute(
                "AllGather", mybir.AluOpType.bypass, replica_groups=groups,
                ins=[slice2[:]], outs=[tab3[:]])
            emit_layer(2, tab3, wg2, bg2c, False, None)

            # ---- pooling: partial means -> AllReduce
            for half, invc in (("A", invcA), ("B", invcB)):
                m_sb = workp.tile([P, F], F32, tag=f"m{half}")
                nc.vector.tensor_scalar(
                    out=m_sb[:], in0=pool_ps[half][:], scalar1=invc[:, 0:1],
                    scalar2=None, op0=mybir.AluOpType.mult)
                base = 0 if half == "A" else P
                nc.sync.dma_start(out=pool_in[base:base + P, :], in_=m_sb[:])
            nc.gpsimd.collective_compute(
                "AllReduce", mybir.AluOpType.add, replica_groups=groups,
                ins=[pool_in[:]], outs=[pool_out[:]])

            # ---- head (redundant on every core)
            g1T = {}
            for hi, half in enumerate(("A", "B")):
                m_sb = workp.tile([P, F], F32, tag=f"mf{half}")
                nc.sync.dma_start(out=m_sb[:], in_=pool_out[hi * P:(hi + 1) * P, :])
                mT_ps = phT.tile([P, F], F32, space="PSUM", tag="hT")
                nc.tensor.transpose(out=mT_ps[:], in_=m_sb[:], identity=ident[:])
                mT = workp.tile([P, F], F32, tag=f"mT{half}")
                nc.vector.tensor_copy(out=mT[:], in_=mT_ps[:])
                for h in range(NHID // P):
                    g_ps = pagg.tile([P, P], F32, space="PSUM", tag="agg")
                    nc.tensor.matmul(out=g_ps[:], lhsT=wh1[:, h * P:(h + 1) * P],
                                     rhs=mT[:], start=True, stop=True)
                    gt = workp.tile([P, P], F32, tag=f"g1T{half}{h}")
                    nc.scalar.activation(out=gt[:], in_=g_ps[:],
                                         func=mybir.ActivationFunctionType.Relu,
                                         bias=bh1c[:, h:h + 1])
                    g1T[(half, h)] = gt
            for hi, half in enumerate(("A", "B")):
                o_ps = pagg.tile([P, NOUT], F32, space="PSUM", tag="agg")
                for h in range(NHID // P):
                    nc.tensor.matmul(out=o_ps[:], lhsT=g1T[(half, h)][:],
                                     rhs=wh2[:, h * NOUT:(h + 1) * NOUT],
                                     start=(h == 0), stop=(h == NHID // P - 1))
                o_sb = workp.tile([P, NOUT], F32, tag=f"o{half}")
                nc.vector.tensor_add(out=o_sb[:], in0=o_ps[:], in1=bh2rep[:])
                nc.sync.dma_start(out=out_d[hi * P:(hi + 1) * P, :], in_=o_sb[:])

    nc.compile()
    return nc


_CACHE = {}


def run(cfg, inputs):
    in_maps, meta = preprocess(cfg, **inputs)
    key = (cfg.N, tuple(meta["NTBLO"]), tuple(meta["NTBHI"]),
           meta["NGLO"], meta["NGHI"])
    if key not in _CACHE:
        _CACHE[key] = build_program(cfg, meta)
    nc = _CACHE[key]
    res = run_bass_kernel_spmd(nc, in_maps, core_ids=list(range(NCORES)))
    return res.results[0]["out"].astype(np.float32)


def kernel(**inputs):
    return run(FULL, inputs)



# revision 6
# speedup vs baseline: 1.9489x; 1.9489x over previous
"""GCN+GIN graph encoder on 8 Trainium2 NeuronCores (Bass/Tile).

Math (reference):
  GCNConv:  h = relu(segsum_dst(norm_e * (x@W0)[src]) + b0),
            norm_e = dinv[src]*dinv[dst] over edges+self-loops,
            dinv = rsqrt(deg incl self-loop)
  GIN x2:   h = relu((h + segsum_dst(h[src])) @ Wg + bg)
  pool:     m = segment_mean(h, batch) -> relu(m@Wh1+bh1)@Wh2+bh2

Distribution: nodes (and their in-edges) sharded contiguously over 8 cores.
Per layer each core aggregates messages for its own dst nodes by gathering
rows of a replicated node-feature table (dma_gather on 4 SWDGE queues),
reducing edge tiles with one-hot selection matrices on the TensorEngine,
applying the layer linear transform W-stationary in feat-major, then
transposing back to node-major.  Tables are re-replicated between layers
with an AllGather; pooled partials are combined with an AllReduce and the
small MLP head is computed redundantly on every core.

The per-layer work runs inside a single For_i hardware loop over the 49
dst blocks (every block padded to the same per-stream tile count), so the
static program stays ~300 instructions instead of ~12k fully unrolled —
the dominant cost in this harness is per-exec NEFF staging, which scales
with static instruction count.

Aggregation identity per dst block b (128 dst nodes):
  aggT[f, d] = sum_e msg[e, f] * sel[e, d],  sel[e, d] = (doff[e] == d) * val[e]
computed as matmul(lhsT=msg_tile[128e, 128f], rhs=sel[128e, 128d]) accumulated
in PSUM over the block's edge tiles.  GCN folds dinv[src] into the table rows
(host-prescaled x) and dinv[dst] into val; GIN uses val=1 and a self-loop edge
supplies the "+h" term.  Pad edge slots carry doff=-1 -> zero contribution.
"""
import sys

sys.path.insert(0, '/opt/trn_rl_repo')

import numpy as np

import concourse.bass as bass
import concourse.bacc as bacc
import concourse.mybir as mybir
import concourse.tile as tile
from concourse.bass import ds, ts
from concourse.bass_utils import run_bass_kernel_spmd
from concourse.masks import make_identity

F32 = mybir.dt.float32
I16 = mybir.dt.int16
P = 128
NCORES = 8
GMAX = 1024                 # max rows per dma_gather (single_packet limit)
NQ = 4                      # SWDGE queues


class Cfg:
    def __init__(self, N, E, G, F, NHID, NOUT, NPN):
        self.N = N            # real nodes
        self.E = E            # edges (no self loops)
        self.G = G            # graphs
        self.F = F            # feature/hidden width (128)
        self.NHID = NHID
        self.NOUT = NOUT
        self.NPN = NPN        # real nodes per core
        assert NPN * NCORES >= N > NPN * (NCORES - 1)
        self.NPC = ((NPN + P - 1) // P) * P   # padded nodes per core
        self.NBLK = self.NPC // P
        self.NPAD = self.NPC * NCORES
        self.NHALF = self.NPAD // 2
        assert self.NHALF < 32768
        assert G == 2 * P


FULL = Cfg(N=50000, E=800000, G=256, F=128, NHID=256, NOUT=128, NPN=6250)


# ---------------------------------------------------------------- host prep
def preprocess(cfg, x, edge_index, batch, W0, b0, Wg1, bg1, Wg2, bg2,
               Wh1, bh1, Wh2, bh2):
    N, G, F = cfg.N, cfg.G, cfg.F
    NPN, NPC, NBLK, NHALF = cfg.NPN, cfg.NPC, cfg.NBLK, cfg.NHALF

    src = np.asarray(edge_index[0], dtype=np.int64)
    dst = np.asarray(edge_index[1], dtype=np.int64)
    batch = np.asarray(batch, dtype=np.int64)
    loop = np.arange(N, dtype=np.int64)
    s_all = np.concatenate([src, loop])
    d_all = np.concatenate([dst, loop])

    deg = np.bincount(d_all, minlength=N).astype(np.float64)
    dinv = (1.0 / np.sqrt(np.maximum(deg, 1.0))).astype(np.float32)

    def tabidx(n):
        c = n // NPN
        return c * NPC + (n - c * NPN)

    sidx = tabidx(s_all).astype(np.int64)
    c_e = d_all // NPN
    loc = d_all - c_e * NPN
    b_e = loc // P
    off_e = loc % P
    gblk = c_e * NBLK + b_e                      # global dst block id
    val_e = dinv[d_all].astype(np.float32)      # GCN dst scaling

    NGB = NCORES * NBLK
    streams = {}
    for name, mask in (("lo", sidx < NHALF), ("hi", sidx >= NHALF)):
        sg = gblk[mask]
        si = sidx[mask] - (0 if name == "lo" else NHALF)
        sof = off_e[mask]
        sva = val_e[mask]
        order = np.argsort(sg, kind="stable")
        sg, si, sof, sva = sg[order], si[order], sof[order], sva[order]
        cnt = np.bincount(sg, minlength=NGB)
        # uniform per-block tile count (same For_i body for every block/core)
        NT = int(np.ceil(cnt.max() / P))
        rows_blk = NT * P
        rows_core = NBLK * rows_blk
        starts = np.zeros(NGB, dtype=np.int64)
        starts[1:] = np.cumsum(cnt)[:-1]
        rank = np.arange(len(sg)) - np.repeat(starts, cnt)
        c_of = sg // NBLK
        b_of = sg % NBLK
        pos = c_of * rows_core + b_of * rows_blk + rank
        tot = NCORES * rows_core
        idx_arr = np.zeros(tot, dtype=np.int32)
        doff_arr = np.full(tot, -1.0, dtype=np.float32)
        val_arr = np.zeros(tot, dtype=np.float32)
        idx_arr[pos] = si
        doff_arr[pos] = sof
        val_arr[pos] = sva
        idx_arr = idx_arr.reshape(NCORES, NBLK, rows_blk)
        # gather chunk sizes within a block: full 1024s then the remainder
        chunks = [GMAX] * (rows_blk // GMAX)
        if rows_blk % GMAX:
            chunks.append(rows_blk % GMAX)
        cols_blk = rows_blk // 16
        wrapped = np.zeros((NCORES, 16, NBLK * cols_blk), dtype=np.int16)
        for b in range(NBLK):
            a = 0
            cc = b * cols_blk
            for sz in chunks:
                wrapped[:, :, cc:cc + sz // 16] = (
                    idx_arr[:, b, a:a + sz].reshape(NCORES, sz // 16, 16)
                    .swapaxes(1, 2))
                a += sz
                cc += sz // 16
        T = NBLK * NT
        doff2 = doff_arr.reshape(NCORES, T, P).transpose(0, 2, 1).copy()
        val2 = val_arr.reshape(NCORES, T, P).transpose(0, 2, 1).copy()
        streams[name] = dict(NT=NT, T=T, chunks=chunks, cols_blk=cols_blk,
                             idx=wrapped, doff=doff2, val=val2)

    # per-core node-feature slice, pre-scaled by dinv (GCN source scaling)
    xs = np.zeros((NCORES, NPC, F), dtype=np.float32)
    x = np.asarray(x, dtype=np.float32)
    for c in range(NCORES):
        lo_n = c * NPN
        hi_n = min(N, (c + 1) * NPN)
        n = hi_n - lo_n
        xs[c, :n] = x[lo_n:hi_n] * dinv[lo_n:hi_n, None]

    # pooling metadata: batch id per node, block-column-major, pad=-1
    cnt_g = np.bincount(batch, minlength=G).astype(np.float32)
    invc = (1.0 / np.maximum(cnt_g, 1.0)).astype(np.float32)
    bat = np.full((NCORES, P, NBLK), -1.0, dtype=np.float32)
    for c in range(NCORES):
        lo_n = c * NPN
        hi_n = min(N, (c + 1) * NPN)
        n = hi_n - lo_n
        colmaj = np.full(NPC, -1.0, dtype=np.float32)
        colmaj[:n] = batch[lo_n:hi_n].astype(np.float32)
        bat[c] = colmaj.reshape(NBLK, P).T

    iota = np.broadcast_to(np.arange(P, dtype=np.float32), (P, P)).copy()
    iotaG = np.broadcast_to(np.arange(G, dtype=np.float32), (P, G)).copy()
    invc_rep = np.broadcast_to(invc, (P, G)).copy()

    common = dict(
        iota=iota, iotaG=iotaG, invc_rep=invc_rep,
        w0=np.asarray(W0, np.float32), wg1=np.asarray(Wg1, np.float32),
        wg2=np.asarray(Wg2, np.float32),
        b0c=np.asarray(b0, np.float32).reshape(P, 1).copy(),
        bg1c=np.asarray(bg1, np.float32).reshape(P, 1).copy(),
        bg2c=np.asarray(bg2, np.float32).reshape(P, 1).copy(),
        wh1=np.asarray(Wh1, np.float32),
        bh1c=np.asarray(bh1, np.float32).reshape(2, P).T.copy(),  # [128,2]
        wh2=np.asarray(Wh2, np.float32),
        bh2c=np.asarray(bh2, np.float32).reshape(P, 1).copy(),
    )
    in_maps = []
    for c in range(NCORES):
        m = dict(common)
        m.update(
            xs=xs[c],
            idxlo=streams["lo"]["idx"][c], idxhi=streams["hi"]["idx"][c],
            dofflo=streams["lo"]["doff"][c], doffhi=streams["hi"]["doff"][c],
            vallo=streams["lo"]["val"][c], valhi=streams["hi"]["val"][c],
            bat=bat[c],
        )
        in_maps.append(m)
    meta = dict(NTLO=streams["lo"]["NT"], CHLO=streams["lo"]["chunks"],
                NTHI=streams["hi"]["NT"], CHHI=streams["hi"]["chunks"])
    return in_maps, meta


# ---------------------------------------------------------------- program
def build_program(cfg, meta):
    NPC, NBLK, NPAD, NHALF = cfg.NPC, cfg.NBLK, cfg.NPAD, cfg.NHALF
    F, NHID, NOUT, G = cfg.F, cfg.NHID, cfg.NOUT, cfg.G
    NTLO, CHLO = meta["NTLO"], meta["CHLO"]
    NTHI, CHHI = meta["NTHI"], meta["CHHI"]
    TLO, THI = NBLK * NTLO, NBLK * NTHI
    CLO, CHI = TLO * 8, THI * 8          # idx cols (= rows/16) per core

    nc = bacc.Bacc(None, target_bir_lowering=False, debug=True,
                   num_devices=NCORES, num_swdge_queues=NQ)

    def din(name, shape, dt=F32):
        return nc.declare_dram_parameter(name, list(shape), dt, isOutput=False)

    xs_d = din("xs", [NPC, F])
    idxlo_d = din("idxlo", [16, CLO], I16)
    idxhi_d = din("idxhi", [16, CHI], I16)
    dofflo_d = din("dofflo", [P, TLO])
    doffhi_d = din("doffhi", [P, THI])
    vallo_d = din("vallo", [P, TLO])
    valhi_d = din("valhi", [P, THI])
    iota_d = din("iota", [P, P])
    iotaG_d = din("iotaG", [P, G])
    invc_d = din("invc_rep", [P, G])
    w0_d = din("w0", [F, F]); wg1_d = din("wg1", [F, F]); wg2_d = din("wg2", [F, F])
    b0c_d = din("b0c", [P, 1]); bg1c_d = din("bg1c", [P, 1]); bg2c_d = din("bg2c", [P, 1])
    wh1_d = din("wh1", [F, NHID]); bh1c_d = din("bh1c", [P, 2])
    wh2_d = din("wh2", [NHID, NOUT]); bh2c_d = din("bh2c", [P, 1])
    bat_d = din("bat", [P, NBLK])
    out_d = nc.declare_dram_parameter("out", [G, NOUT], F32, isOutput=True)

    slice0 = nc.dram_tensor("slice0", [NPC, F], F32)
    slice1 = nc.dram_tensor("slice1", [NPC, F], F32)
    slice2 = nc.dram_tensor("slice2", [NPC, F], F32)
    tab1 = nc.dram_tensor("tab1", [NPAD, F], F32)
    tab2 = nc.dram_tensor("tab2", [NPAD, F], F32)
    tab3 = nc.dram_tensor("tab3", [NPAD, F], F32)
    pool_in = nc.dram_tensor("pool_in", [P, G], F32)
    pool_out = nc.dram_tensor("pool_out", [P, G], F32, addr_space="Shared")
    groups = [list(range(NCORES))]

    with tile.TileContext(nc) as tc:
        with (
            tc.tile_pool(name="const", bufs=1) as constp,
            tc.tile_pool(name="meta", bufs=1) as metap,
            tc.tile_pool(name="msg", bufs=2) as msgp,
            tc.tile_pool(name="sel", bufs=4) as selp,
            tc.tile_pool(name="work", bufs=4) as workp,
            tc.tile_pool(name="pagg", bufs=1, space="PSUM") as pagg,
            tc.tile_pool(name="phT", bufs=1, space="PSUM") as phT,
            tc.tile_pool(name="ptr", bufs=1, space="PSUM") as ptr,
            tc.tile_pool(name="ppool", bufs=1, space="PSUM") as ppool,
            tc.tile_pool(name="phead", bufs=1, space="PSUM") as phead,
        ):
            # ---- constants / metadata to SBUF
            ident = constp.tile([P, P], F32)
            make_identity(nc, ident[:])

            def load(t_shape, dram, dt=F32, pool=metap):
                nm = f"sb_{dram.name}"
                t = pool.tile(list(t_shape), dt, name=nm, tag=nm)
                nc.sync.dma_start(out=t[:], in_=dram[:])
                return t

            # idx tables: ship [16, C], replicate to 128 partitions on device
            idxlo = metap.tile([P, CLO], I16, tag="idxlo")
            idxhi = metap.tile([P, CHI], I16, tag="idxhi")
            for k in range(8):
                nc.sync.dma_start(out=idxlo[16 * k:16 * (k + 1), :], in_=idxlo_d[:])
                nc.sync.dma_start(out=idxhi[16 * k:16 * (k + 1), :], in_=idxhi_d[:])
            iota = load([P, P], iota_d, pool=constp)
            iotaG = load([P, G], iotaG_d, pool=constp)
            invc_rep = load([P, G], invc_d, pool=constp)
            dofflo = load([P, TLO], dofflo_d)
            doffhi = load([P, THI], doffhi_d)
            vallo = load([P, TLO], vallo_d)
            valhi = load([P, THI], valhi_d)
            w0 = load([F, F], w0_d, pool=constp)
            wg1 = load([F, F], wg1_d, pool=constp)
            wg2 = load([F, F], wg2_d, pool=constp)
            b0c = load([P, 1], b0c_d, pool=constp)
            bg1c = load([P, 1], bg1c_d, pool=constp)
            bg2c = load([P, 1], bg2c_d, pool=constp)
            wh1 = load([F, NHID], wh1_d, pool=constp)
            bh1c = load([P, 2], bh1c_d, pool=constp)
            wh2 = constp.tile([P, (NHID // P) * NOUT], F32)
            for h in range(NHID // P):
                nc.sync.dma_start(out=wh2[:, h * NOUT:(h + 1) * NOUT],
                                  in_=wh2_d[h * P:(h + 1) * P, :])
            bh2c = load([P, 1], bh2c_d, pool=constp)
            bat = load([P, NBLK], bat_d, pool=constp)

            # stage xs -> slice0 (collectives need internal tensors)
            nc.sync.dma_start(out=slice0[:], in_=xs_d[:])
            nc.gpsimd.collective_compute(
                "AllGather", mybir.AluOpType.bypass, replica_groups=groups,
                ins=[slice0[:]], outs=[tab1[:]])

            pool_acc = constp.tile([P, G], F32, tag="pool_acc")

            def emit_layer(L, tab, W_sb, bias_col, use_val, out_slice):
                stream_info = [
                    ("lo", NTLO, CHLO, idxlo, dofflo, vallo, tab[0:NHALF, :]),
                    ("hi", NTHI, CHHI, idxhi, doffhi, valhi, tab[NHALF:NPAD, :]),
                ]
                with tc.For_i(0, NBLK, name=f"layer{L}") as i:
                    bufs = {}
                    qn = 0
                    for sname, NT, CH, idx_sb, _, _, tab_ap in stream_info:
                        buf = msgp.tile([P, NT * P], F32, tag=f"buf{sname}")
                        bufs[sname] = buf
                        a = 0       # rows done within block
                        for sz in CH:
                            nc.gpsimd.dma_gather(
                                out_ap=buf[:, a:a + sz].rearrange(
                                    "p (c f) -> p c f", f=F),
                                in_ap=tab_ap,
                                idxs_ap=idx_sb[:, ds(i * (NT * 8) + a // 16,
                                                     sz // 16)],
                                num_idxs=sz, num_idxs_reg=sz,
                                elem_size=F, single_packet=True,
                                queue_num=qn % NQ)
                            qn += 1
                            a += sz
                    agg_ps = pagg.tile([P, F], F32, space="PSUM", tag="agg")
                    ntot = NTLO + NTHI
                    wi = 0
                    for sname, NT, CH, idx_sb, doff_sb, val_sb, tab_ap in stream_info:
                        buf = bufs[sname]
                        for tt in range(NT):
                            sel = selp.tile([P, P], F32, tag=f"sel{sname}{tt % 4}")
                            col = ds(i * NT + tt, 1)
                            if use_val:
                                nc.vector.tensor_scalar(
                                    out=sel[:], in0=iota[:],
                                    scalar1=doff_sb[:, col],
                                    scalar2=val_sb[:, col],
                                    op0=mybir.AluOpType.is_equal,
                                    op1=mybir.AluOpType.mult)
                            else:
                                nc.vector.tensor_scalar(
                                    out=sel[:], in0=iota[:],
                                    scalar1=doff_sb[:, col], scalar2=None,
                                    op0=mybir.AluOpType.is_equal)
                            nc.tensor.matmul(
                                out=agg_ps[:],
                                lhsT=buf[:, tt * F:(tt + 1) * F],
                                rhs=sel[:], start=(wi == 0),
                                stop=(wi == ntot - 1))
                            wi += 1
                    aggT = workp.tile([P, F], F32, tag="aggT")
                    nc.vector.tensor_copy(out=aggT[:], in_=agg_ps[:])
                    hT_ps = phT.tile([P, F], F32, space="PSUM", tag="hT")
                    nc.tensor.matmul(out=hT_ps[:], lhsT=W_sb[:], rhs=aggT[:],
                                     start=True, stop=True)
                    hT = workp.tile([P, F], F32, tag="hT_sb")
                    nc.scalar.activation(out=hT[:], in_=hT_ps[:],
                                         func=mybir.ActivationFunctionType.Relu,
                                         bias=bias_col[:, 0:1])
                    h_ps = ptr.tile([P, F], F32, space="PSUM", tag="tr")
                    nc.tensor.transpose(out=h_ps[:], in_=hT[:], identity=ident[:])
                    h_sb = workp.tile([P, F], F32, tag="h_sb")
                    nc.vector.tensor_copy(out=h_sb[:], in_=h_ps[:])
                    if out_slice is not None:
                        nc.sync.dma_start(out=out_slice[ts(i, P), :], in_=h_sb[:])
                    else:
                        # pool: one-hot [node -> graph] and accumulate [F, G]
                        selg = selp.tile([P, G], F32, tag="selg")
                        nc.vector.tensor_scalar(
                            out=selg[:], in0=iotaG[:],
                            scalar1=bat[:, ds(i, 1)], scalar2=None,
                            op0=mybir.AluOpType.is_equal)
                        pmm = ppool.tile([P, G], F32, space="PSUM", tag="pmm")
                        nc.tensor.matmul(out=pmm[:], lhsT=h_sb[:], rhs=selg[:],
                                         start=True, stop=True)
                        nc.vector.tensor_add(out=pool_acc[:], in0=pool_acc[:],
                                             in1=pmm[:])

            emit_layer(0, tab1, w0, b0c, True, slice1)
            nc.gpsimd.collective_compute(
                "AllGather", mybir.AluOpType.bypass, replica_groups=groups,
                ins=[slice1[:]], outs=[tab2[:]])
            emit_layer(1, tab2, wg1, bg1c, False, slice2)
            nc.gpsimd.collective_compute(
                "AllGather", mybir.AluOpType.bypass, replica_groups=groups,
                ins=[slice2[:]], outs=[tab3[:]])
            nc.any.memset(pool_acc[:], 0.0)
            emit_layer(2, tab3, wg2, bg2c, False, None)

            # ---- pooling: partial sums [F, G] -> AllReduce -> mean
            nc.sync.dma_start(out=pool_in[:], in_=pool_acc[:])
            nc.gpsimd.collective_compute(
                "AllReduce", mybir.AluOpType.add, replica_groups=groups,
                ins=[pool_in[:]], outs=[pool_out[:]])
            mT = workp.tile([P, G], F32, tag="mT")     # [F, G] mean, feat-major
            nc.sync.dma_start(out=mT[:], in_=pool_out[:])
            nc.vector.tensor_mul(out=mT[:], in0=mT[:], in1=invc_rep[:])

            # ---- head (redundant on every core), all graph-minor [*, G]
            g1T = []
            for h in range(NHID // P):
                g_ps = phead.tile([P, G], F32, space="PSUM", tag=f"ghead{h}")
                nc.tensor.matmul(out=g_ps[:], lhsT=wh1[:, h * P:(h + 1) * P],
                                 rhs=mT[:], start=True, stop=True)
                gt = workp.tile([P, G], F32, tag=f"g1T{h}")
                nc.scalar.activation(out=gt[:], in_=g_ps[:],
                                     func=mybir.ActivationFunctionType.Relu,
                                     bias=bh1c[:, h:h + 1])
                g1T.append(gt)
            o_ps = phead.tile([P, G], F32, space="PSUM", tag="ohead")
            for h in range(NHID // P):
                nc.tensor.matmul(out=o_ps[:], lhsT=wh2[:, h * NOUT:(h + 1) * NOUT],
                                 rhs=g1T[h][:], start=(h == 0),
                                 stop=(h == NHID // P - 1))
            outT = workp.tile([P, G], F32, tag="outT")   # [NOUT, G]
            nc.vector.tensor_scalar(out=outT[:], in0=o_ps[:],
                                    scalar1=bh2c[:, 0:1], scalar2=None,
                                    op0=mybir.AluOpType.add)
            for gc in range(G // P):
                tr_ps = ptr.tile([P, P], F32, space="PSUM", tag="tr")
                nc.tensor.transpose(out=tr_ps[:], in_=outT[:, gc * P:(gc + 1) * P],
                                    identity=ident[:])
                o_sb = workp.tile([P, NOUT], F32, tag=f"o{gc}")
                nc.vector.tensor_copy(out=o_sb[:], in_=tr_ps[:])
                nc.sync.dma_start(out=out_d[gc * P:(gc + 1) * P, :], in_=o_sb[:])

    nc.compile()
    return nc


_CACHE = {}


def run(cfg, inputs):
    in_maps, meta = preprocess(cfg, **inputs)
    key = (cfg.N, meta["NTLO"], meta["NTHI"])
    if key not in _CACHE:
        _CACHE[key] = build_program(cfg, meta)
    nc = _CACHE[key]
    res = run_bass_kernel_spmd(nc, in_maps, core_ids=list(range(NCORES)))
    return res.results[0]["out"].astype(np.float32)


def kernel(**inputs):
    return run(FULL, inputs)


# revision 9
# speedup vs baseline: 4.8942x; 2.5113x over previous
"""GCN+GIN graph encoder on 8 Trainium2 NeuronCores (Bass/Tile).

Math (reference):
  GCNConv:  h = relu(segsum_dst(norm_e * (x@W0)[src]) + b0),
            norm_e = dinv[src]*dinv[dst] over edges+self-loops,
            dinv = rsqrt(deg incl self-loop)
  GIN x2:   h = relu((h + segsum_dst(h[src])) @ Wg + bg)
  pool:     m = segment_mean(h, batch) -> relu(m@Wh1+bh1)@Wh2+bh2

Distribution: nodes (and their in-edges) sharded contiguously over 8 cores.
Per layer each core aggregates messages for its own dst nodes by gathering
rows of a replicated bf16 node-feature table (dma_gather on 4 SWDGE queues),
reducing edge tiles with one-hot selection matrices on the TensorEngine,
applying the layer linear transform W-stationary in feat-major, then
transposing back to node-major.  Tables are re-replicated between layers
with an AllGather; pooled partials are combined with an AllReduce and the
small MLP head is computed redundantly on every core.

Harness-cost driven design:
 * per-exec NEFF staging scales with STATIC instruction count -> each
   layer's per-block work runs in a single For_i hardware loop (every
   block padded to the same per-stream tile count), keeping the static
   program ~700 instructions instead of ~12k fully unrolled.
 * per-exec input shipping costs ~25 ms/MB and ~10 ms/array -> all
   per-core inputs are packed into ONE uint8 blob (bf16/int8/int16
   sections, bitcast-viewed on device), with edge streams in
   idx:int16 / doff:int8 / val:bf16 and features in bf16.

Aggregation identity per dst block b (128 dst nodes):
  aggT[f, d] = sum_e msg[e, f] * sel[e, d],  sel[e, d] = (doff[e] == d) * val[e]
computed as matmul(lhsT=msg_tile[128e, 128f], rhs=sel[128e, 128d]) accumulated
in PSUM over the block's edge tiles.  GCN folds dinv[src] into the table rows
(host-prescaled x) and dinv[dst] into val; GIN uses val=1 and a self-loop edge
supplies the "+h" term.  Pad edge slots carry doff=-1 -> zero contribution.
"""
import sys

sys.path.insert(0, '/opt/trn_rl_repo')

import numpy as np
import ml_dtypes

import concourse.bass as bass
import concourse.bacc as bacc
import concourse.mybir as mybir
import concourse.tile as tile
from concourse.bass import ds, ts
from concourse.bass_utils import run_bass_kernel_spmd
from concourse.masks import make_identity

F32 = mybir.dt.float32
BF16 = mybir.dt.bfloat16
I16 = mybir.dt.int16
I8 = mybir.dt.int8
U8 = mybir.dt.uint8
BF = ml_dtypes.bfloat16
P = 128
NCORES = 8
GMAX = 1024                 # max rows per dma_gather (single_packet limit)
NQ = 4                      # SWDGE queues
ALIGN = 512


class Cfg:
    def __init__(self, N, E, G, F, NHID, NOUT, NPN):
        self.N = N            # real nodes
        self.E = E            # edges (no self loops)
        self.G = G            # graphs
        self.F = F            # feature/hidden width (128)
        self.NHID = NHID
        self.NOUT = NOUT
        self.NPN = NPN        # real nodes per core
        assert NPN * NCORES >= N > NPN * (NCORES - 1)
        self.NPC = ((NPN + P - 1) // P) * P   # padded nodes per core
        self.NBLK = self.NPC // P
        self.NPAD = self.NPC * NCORES
        self.NHALF = self.NPAD // 2
        assert self.NHALF < 32768
        assert G == 2 * P


FULL = Cfg(N=50000, E=800000, G=256, F=128, NHID=256, NOUT=128, NPN=6250)


# ---------------------------------------------------------------- host prep
def preprocess(cfg, x, edge_index, batch, W0, b0, Wg1, bg1, Wg2, bg2,
               Wh1, bh1, Wh2, bh2):
    N, G, F = cfg.N, cfg.G, cfg.F
    NPN, NPC, NBLK, NHALF = cfg.NPN, cfg.NPC, cfg.NBLK, cfg.NHALF

    src = np.asarray(edge_index[0], dtype=np.int64)
    dst = np.asarray(edge_index[1], dtype=np.int64)
    batch = np.asarray(batch, dtype=np.int64)
    loop = np.arange(N, dtype=np.int64)
    s_all = np.concatenate([src, loop])
    d_all = np.concatenate([dst, loop])

    deg = np.bincount(d_all, minlength=N).astype(np.float64)
    dinv = (1.0 / np.sqrt(np.maximum(deg, 1.0))).astype(np.float32)

    def tabidx(n):
        c = n // NPN
        return c * NPC + (n - c * NPN)

    sidx = tabidx(s_all).astype(np.int64)
    c_e = d_all // NPN
    loc = d_all - c_e * NPN
    b_e = loc // P
    off_e = loc % P
    gblk = c_e * NBLK + b_e                      # global dst block id
    val_e = dinv[d_all].astype(np.float32)      # GCN dst scaling

    NGB = NCORES * NBLK
    streams = {}
    for name, mask in (("lo", sidx < NHALF), ("hi", sidx >= NHALF)):
        sg = gblk[mask]
        si = sidx[mask] - (0 if name == "lo" else NHALF)
        sof = off_e[mask]
        sva = val_e[mask]
        order = np.argsort(sg, kind="stable")
        sg, si, sof, sva = sg[order], si[order], sof[order], sva[order]
        cnt = np.bincount(sg, minlength=NGB)
        # uniform per-block tile count (same For_i body for every block/core)
        NT = int(np.ceil(cnt.max() / P))
        rows_blk = NT * P
        rows_core = NBLK * rows_blk
        starts = np.zeros(NGB, dtype=np.int64)
        starts[1:] = np.cumsum(cnt)[:-1]
        rank = np.arange(len(sg)) - np.repeat(starts, cnt)
        c_of = sg // NBLK
        b_of = sg % NBLK
        pos = c_of * rows_core + b_of * rows_blk + rank
        tot = NCORES * rows_core
        idx_arr = np.zeros(tot, dtype=np.int32)
        doff_arr = np.full(tot, -1, dtype=np.int8)
        val_arr = np.zeros(tot, dtype=np.float32)
        idx_arr[pos] = si
        doff_arr[pos] = sof
        val_arr[pos] = sva
        idx_arr = idx_arr.reshape(NCORES, NBLK, rows_blk)
        # gather chunk sizes within a block: full 1024s then the remainder
        chunks = [GMAX] * (rows_blk // GMAX)
        if rows_blk % GMAX:
            chunks.append(rows_blk % GMAX)
        cols_blk = rows_blk // 16
        wrapped = np.zeros((NCORES, 16, NBLK * cols_blk), dtype=np.int16)
        for b in range(NBLK):
            a = 0
            cc = b * cols_blk
            for sz in chunks:
                wrapped[:, :, cc:cc + sz // 16] = (
                    idx_arr[:, b, a:a + sz].reshape(NCORES, sz // 16, 16)
                    .swapaxes(1, 2))
                a += sz
                cc += sz // 16
        T = NBLK * NT
        doff2 = doff_arr.reshape(NCORES, T, P).transpose(0, 2, 1).copy()
        val2 = val_arr.reshape(NCORES, T, P).transpose(0, 2, 1).astype(BF)
        streams[name] = dict(NT=NT, T=T, chunks=chunks,
                             idx=wrapped, doff=doff2, val=val2)

    # per-core node-feature slice, pre-scaled by dinv (GCN source scaling)
    xs = np.zeros((NCORES, NPC, F), dtype=BF)
    x = np.asarray(x, dtype=np.float32)
    for c in range(NCORES):
        lo_n = c * NPN
        hi_n = min(N, (c + 1) * NPN)
        n = hi_n - lo_n
        xs[c, :n] = (x[lo_n:hi_n] * dinv[lo_n:hi_n, None]).astype(BF)

    # pooling metadata: batch id per node, block-column-major, pad=-1
    cnt_g = np.bincount(batch, minlength=G).astype(np.float32)
    invc = (1.0 / np.maximum(cnt_g, 1.0)).astype(np.float32)
    bat = np.full((NCORES, P, NBLK), -1.0, dtype=BF)
    for c in range(NCORES):
        lo_n = c * NPN
        hi_n = min(N, (c + 1) * NPN)
        n = hi_n - lo_n
        colmaj = np.full(NPC, -1.0, dtype=np.float32)
        colmaj[:n] = batch[lo_n:hi_n].astype(np.float32)
        bat[c] = colmaj.reshape(NBLK, P).T.astype(BF)

    # weights bf16; wh2 packed [P, 2*NOUT] (chunk h at cols h*NOUT)
    wh2 = np.asarray(Wh2, np.float32)
    wh2pack = np.concatenate([wh2[0:P, :], wh2[P:2 * P, :]], axis=1).astype(BF)
    # f32 bias columns [P, 6]: b0, bg1, bg2, bh1_0, bh1_1, bh2
    bh1 = np.asarray(bh1, np.float32)
    bcols = np.stack([
        np.asarray(b0, np.float32), np.asarray(bg1, np.float32),
        np.asarray(bg2, np.float32), bh1[0:P].reshape(P), bh1[P:2 * P].reshape(P),
        np.asarray(bh2, np.float32)], axis=1).copy()

    common = [
        ("w0", np.asarray(W0, np.float32).astype(BF)),
        ("wg1", np.asarray(Wg1, np.float32).astype(BF)),
        ("wg2", np.asarray(Wg2, np.float32).astype(BF)),
        ("wh1", np.asarray(Wh1, np.float32).astype(BF)),
        ("wh2pack", wh2pack),
        ("bcols", bcols),
        ("invc", invc.reshape(1, G)),
    ]

    # ---- pack per-core blobs
    sections = [
        ("xs", None), ("idxlo", None), ("idxhi", None),
        ("dofflo", None), ("doffhi", None), ("vallo", None), ("valhi", None),
        ("bat", None),
    ] + common
    percore = {
        "xs": xs,
        "idxlo": streams["lo"]["idx"], "idxhi": streams["hi"]["idx"],
        "dofflo": streams["lo"]["doff"], "doffhi": streams["hi"]["doff"],
        "vallo": streams["lo"]["val"], "valhi": streams["hi"]["val"],
        "bat": bat,
    }
    offs, off = {}, 0
    for nm, arr in sections:
        a = percore[nm][0] if arr is None else arr
        offs[nm] = off
        off += (a.nbytes + ALIGN - 1) // ALIGN * ALIGN
    BLOB = off
    blobs = np.zeros((NCORES, BLOB), np.uint8)
    for nm, arr in sections:
        for c in range(NCORES):
            a = percore[nm][c] if arr is None else arr
            raw = np.frombuffer(np.ascontiguousarray(a).tobytes(), np.uint8)
            blobs[c, offs[nm]:offs[nm] + raw.size] = raw

    in_maps = [dict(blob=blobs[c:c + 1]) for c in range(NCORES)]
    meta = dict(NTLO=streams["lo"]["NT"], CHLO=streams["lo"]["chunks"],
                NTHI=streams["hi"]["NT"], CHHI=streams["hi"]["chunks"],
                BLOB=BLOB, offs=offs)
    return in_maps, meta


# ---------------------------------------------------------------- program
def build_program(cfg, meta):
    NPC, NBLK, NPAD, NHALF = cfg.NPC, cfg.NBLK, cfg.NPAD, cfg.NHALF
    F, NHID, NOUT, G = cfg.F, cfg.NHID, cfg.NOUT, cfg.G
    NTLO, CHLO = meta["NTLO"], meta["CHLO"]
    NTHI, CHHI = meta["NTHI"], meta["CHHI"]
    TLO, THI = NBLK * NTLO, NBLK * NTHI
    CLO, CHI = TLO * 8, THI * 8          # idx cols (= rows/16) per core
    BLOB, offs = meta["BLOB"], meta["offs"]

    nc = bacc.Bacc(None, target_bir_lowering=False, debug=True,
                   num_devices=NCORES, num_swdge_queues=NQ)

    blob_d = nc.declare_dram_parameter("blob", [1, BLOB], U8, isOutput=False)
    out_d = nc.declare_dram_parameter("out", [G, NOUT], F32, isOutput=True)

    def view(nm, dt, rows, cols):
        esz = mybir.dt.size(dt)
        bc = blob_d.bitcast(dt)
        s = offs[nm] // esz
        return bc[0:1, s:s + rows * cols].rearrange("o (r c) -> (o r) c", c=cols)

    slice0 = nc.dram_tensor("slice0", [NPC, F], BF16)
    slice1 = nc.dram_tensor("slice1", [NPC, F], BF16)
    slice2 = nc.dram_tensor("slice2", [NPC, F], BF16)
    tab1 = nc.dram_tensor("tab1", [NPAD, F], BF16)
    tab2 = nc.dram_tensor("tab2", [NPAD, F], BF16)
    tab3 = nc.dram_tensor("tab3", [NPAD, F], BF16)
    pool_in = nc.dram_tensor("pool_in", [P, G], F32)
    pool_out = nc.dram_tensor("pool_out", [P, G], F32, addr_space="Shared")
    groups = [list(range(NCORES))]

    with tile.TileContext(nc) as tc:
        with (
            tc.tile_pool(name="const", bufs=1) as constp,
            tc.tile_pool(name="meta", bufs=1) as metap,
            tc.tile_pool(name="msg", bufs=2) as msgp,
            tc.tile_pool(name="sel", bufs=4) as selp,
            tc.tile_pool(name="work", bufs=4) as workp,
            tc.tile_pool(name="pagg", bufs=1, space="PSUM") as pagg,
            tc.tile_pool(name="phT", bufs=1, space="PSUM") as phT,
            tc.tile_pool(name="ptr", bufs=1, space="PSUM") as ptr,
            tc.tile_pool(name="ppool", bufs=1, space="PSUM") as ppool,
            tc.tile_pool(name="phead", bufs=1, space="PSUM") as phead,
        ):
            # ---- constants / metadata to SBUF
            ident = constp.tile([P, P], F32)
            make_identity(nc, ident[:])
            iota = constp.tile([P, P], BF16, tag="iota")
            nc.gpsimd.iota(iota[:], pattern=[[1, P]], base=0,
                           channel_multiplier=0,
                           allow_small_or_imprecise_dtypes=True)
            iotaG = constp.tile([P, G], BF16, tag="iotaG")
            nc.gpsimd.iota(iotaG[:], pattern=[[1, G]], base=0,
                           channel_multiplier=0,
                           allow_small_or_imprecise_dtypes=True)

            def load(nm, t_shape, dt=BF16, pool=metap):
                t = pool.tile(list(t_shape), dt, name=f"sb_{nm}", tag=f"sb_{nm}")
                nc.sync.dma_start(out=t[:], in_=view(nm, dt, *t_shape))
                return t

            # idx tables: ship [16, C], replicate to 128 partitions on device
            idxlo = metap.tile([P, CLO], I16, tag="idxlo")
            idxhi = metap.tile([P, CHI], I16, tag="idxhi")
            for k in range(8):
                nc.sync.dma_start(out=idxlo[16 * k:16 * (k + 1), :],
                                  in_=view("idxlo", I16, 16, CLO))
                nc.sync.dma_start(out=idxhi[16 * k:16 * (k + 1), :],
                                  in_=view("idxhi", I16, 16, CHI))
            dofflo8 = load("dofflo", [P, TLO], I8)
            doffhi8 = load("doffhi", [P, THI], I8)
            dofflo = metap.tile([P, TLO], F32, tag="dofflo_f")
            doffhi = metap.tile([P, THI], F32, tag="doffhi_f")
            nc.vector.tensor_copy(out=dofflo[:], in_=dofflo8[:])
            nc.vector.tensor_copy(out=doffhi[:], in_=doffhi8[:])
            vallob = load("vallo", [P, TLO])
            valhib = load("valhi", [P, THI])
            vallo = metap.tile([P, TLO], F32, tag="vallo_f")
            valhi = metap.tile([P, THI], F32, tag="valhi_f")
            nc.vector.tensor_copy(out=vallo[:], in_=vallob[:])
            nc.vector.tensor_copy(out=valhi[:], in_=valhib[:])
            w0 = load("w0", [F, F], pool=constp)
            wg1 = load("wg1", [F, F], pool=constp)
            wg2 = load("wg2", [F, F], pool=constp)
            wh1 = load("wh1", [F, NHID], pool=constp)
            wh2 = load("wh2pack", [P, 2 * NOUT], pool=constp)
            bcols = load("bcols", [P, 6], F32, pool=constp)
            batb = load("bat", [P, NBLK], pool=constp)
            bat = constp.tile([P, NBLK], F32, tag="bat_f")
            nc.vector.tensor_copy(out=bat[:], in_=batb[:])
            # invc broadcast [P, G] via rank-1 outer product ones x invc
            ones1 = constp.tile([1, P], F32, tag="ones1")
            nc.any.memset(ones1[:], 1.0)
            invc_row = load("invc", [1, G], F32, pool=constp)
            invb_ps = phead.tile([P, G], F32, space="PSUM", tag="ghead0")
            nc.tensor.matmul(out=invb_ps[:], lhsT=ones1[:], rhs=invc_row[:],
                             start=True, stop=True)
            invc_rep = constp.tile([P, G], F32, tag="invc_rep")
            nc.vector.tensor_copy(out=invc_rep[:], in_=invb_ps[:])

            # stage xs -> slice0 (collectives need internal tensors)
            nc.sync.dma_start(out=slice0[:], in_=view("xs", BF16, NPC, F))
            nc.gpsimd.collective_compute(
                "AllGather", mybir.AluOpType.bypass, replica_groups=groups,
                ins=[slice0[:]], outs=[tab1[:]])

            pool_acc = constp.tile([P, G], F32, tag="pool_acc")

            def emit_layer(L, tab, W_sb, bias_col, use_val, out_slice):
                stream_info = [
                    ("lo", NTLO, CHLO, idxlo, dofflo, vallo, tab[0:NHALF, :]),
                    ("hi", NTHI, CHHI, idxhi, doffhi, valhi, tab[NHALF:NPAD, :]),
                ]
                with tc.For_i(0, NBLK, name=f"layer{L}") as i:
                    bufs = {}
                    qn = 0
                    for sname, NT, CH, idx_sb, _, _, tab_ap in stream_info:
                        buf = msgp.tile([P, NT * P], BF16, tag=f"buf{sname}")
                        bufs[sname] = buf
                        a = 0       # rows done within block
                        for sz in CH:
                            nc.gpsimd.dma_gather(
                                out_ap=buf[:, a:a + sz].rearrange(
                                    "p (c f) -> p c f", f=F),
                                in_ap=tab_ap,
                                idxs_ap=idx_sb[:, ds(i * (NT * 8) + a // 16,
                                                     sz // 16)],
                                num_idxs=sz, num_idxs_reg=sz,
                                elem_size=F, single_packet=True,
                                queue_num=qn % NQ)
                            qn += 1
                            a += sz
                    agg_ps = pagg.tile([P, F], F32, space="PSUM", tag="agg")
                    ntot = NTLO + NTHI
                    wi = 0
                    for sname, NT, CH, idx_sb, doff_sb, val_sb, tab_ap in stream_info:
                        buf = bufs[sname]
                        for tt in range(NT):
                            sel = selp.tile([P, P], BF16, tag=f"sel{sname}{tt % 4}")
                            col = ds(i * NT + tt, 1)
                            if use_val:
                                nc.vector.tensor_scalar(
                                    out=sel[:], in0=iota[:],
                                    scalar1=doff_sb[:, col],
                                    scalar2=val_sb[:, col],
                                    op0=mybir.AluOpType.is_equal,
                                    op1=mybir.AluOpType.mult)
                            else:
                                nc.vector.tensor_scalar(
                                    out=sel[:], in0=iota[:],
                                    scalar1=doff_sb[:, col], scalar2=None,
                                    op0=mybir.AluOpType.is_equal)
                            nc.tensor.matmul(
                                out=agg_ps[:],
                                lhsT=buf[:, tt * F:(tt + 1) * F],
                                rhs=sel[:], start=(wi == 0),
                                stop=(wi == ntot - 1))
                            wi += 1
                    aggT = workp.tile([P, F], BF16, tag="aggT")
                    nc.vector.tensor_copy(out=aggT[:], in_=agg_ps[:])
                    hT_ps = phT.tile([P, F], F32, space="PSUM", tag="hT")
                    nc.tensor.matmul(out=hT_ps[:], lhsT=W_sb[:], rhs=aggT[:],
                                     start=True, stop=True)
                    hT = workp.tile([P, F], F32, tag="hT_sb")
                    nc.scalar.activation(out=hT[:], in_=hT_ps[:],
                                         func=mybir.ActivationFunctionType.Relu,
                                         bias=bias_col)
                    h_ps = ptr.tile([P, F], F32, space="PSUM", tag="tr")
                    nc.tensor.transpose(out=h_ps[:], in_=hT[:], identity=ident[:])
                    h_sb = workp.tile([P, F], BF16, tag="h_sb")
                    nc.vector.tensor_copy(out=h_sb[:], in_=h_ps[:])
                    if out_slice is not None:
                        nc.sync.dma_start(out=out_slice[ts(i, P), :], in_=h_sb[:])
                    else:
                        # pool: one-hot [node -> graph] and accumulate [F, G]
                        selg = selp.tile([P, G], BF16, tag="selg")
                        nc.vector.tensor_scalar(
                            out=selg[:], in0=iotaG[:],
                            scalar1=bat[:, ds(i, 1)], scalar2=None,
                            op0=mybir.AluOpType.is_equal)
                        pmm = ppool.tile([P, G], F32, space="PSUM", tag="pmm")
                        nc.tensor.matmul(out=pmm[:], lhsT=h_sb[:], rhs=selg[:],
                                         start=True, stop=True)
                        nc.vector.tensor_add(out=pool_acc[:], in0=pool_acc[:],
                                             in1=pmm[:])

            emit_layer(0, tab1, w0, bcols[:, 0:1], True, slice1)
            nc.gpsimd.collective_compute(
                "AllGather", mybir.AluOpType.bypass, replica_groups=groups,
                ins=[slice1[:]], outs=[tab2[:]])
            emit_layer(1, tab2, wg1, bcols[:, 1:2], False, slice2)
            nc.gpsimd.collective_compute(
                "AllGather", mybir.AluOpType.bypass, replica_groups=groups,
                ins=[slice2[:]], outs=[tab3[:]])
            nc.any.memset(pool_acc[:], 0.0)
            emit_layer(2, tab3, wg2, bcols[:, 2:3], False, None)

            # ---- pooling: partial sums [F, G] -> AllReduce -> mean
            nc.sync.dma_start(out=pool_in[:], in_=pool_acc[:])
            nc.gpsimd.collective_compute(
                "AllReduce", mybir.AluOpType.add, replica_groups=groups,
                ins=[pool_in[:]], outs=[pool_out[:]])
            mT = workp.tile([P, G], F32, tag="mT")     # [F, G] mean, feat-major
            nc.sync.dma_start(out=mT[:], in_=pool_out[:])
            nc.vector.tensor_mul(out=mT[:], in0=mT[:], in1=invc_rep[:])
            mTb = workp.tile([P, G], BF16, tag="mTb")
            nc.vector.tensor_copy(out=mTb[:], in_=mT[:])

            # ---- head (redundant on every core), all graph-minor [*, G]
            g1T = []
            for h in range(NHID // P):
                g_ps = phead.tile([P, G], F32, space="PSUM", tag=f"ghead{h}")
                nc.tensor.matmul(out=g_ps[:], lhsT=wh1[:, h * P:(h + 1) * P],
                                 rhs=mTb[:], start=True, stop=True)
                gt = workp.tile([P, G], BF16, tag=f"g1T{h}")
                nc.scalar.activation(out=gt[:], in_=g_ps[:],
                                     func=mybir.ActivationFunctionType.Relu,
                                     bias=bcols[:, 3 + h:4 + h])
                g1T.append(gt)
            o_ps = phead.tile([P, G], F32, space="PSUM", tag="ohead")
            for h in range(NHID // P):
                nc.tensor.matmul(out=o_ps[:], lhsT=wh2[:, h * NOUT:(h + 1) * NOUT],
                                 rhs=g1T[h][:], start=(h == 0),
                                 stop=(h == NHID // P - 1))
            outT = workp.tile([P, G], F32, tag="outT")   # [NOUT, G]
            nc.vector.tensor_scalar(out=outT[:], in0=o_ps[:],
                                    scalar1=bcols[:, 5:6], scalar2=None,
                                    op0=mybir.AluOpType.add)
            for gc in range(G // P):
                tr_ps = ptr.tile([P, P], F32, space="PSUM", tag="tr")
                nc.tensor.transpose(out=tr_ps[:], in_=outT[:, gc * P:(gc + 1) * P],
                                    identity=ident[:])
                o_sb = workp.tile([P, NOUT], F32, tag=f"o{gc}")
                nc.vector.tensor_copy(out=o_sb[:], in_=tr_ps[:])
                nc.sync.dma_start(out=out_d[gc * P:(gc + 1) * P, :], in_=o_sb[:])

    nc.compile()
    return nc


_CACHE = {}


def run(cfg, inputs):
    in_maps, meta = preprocess(cfg, **inputs)
    key = (cfg.N, meta["NTLO"], meta["NTHI"], meta["BLOB"])
    if key not in _CACHE:
        _CACHE[key] = build_program(cfg, meta)
    nc = _CACHE[key]
    res = run_bass_kernel_spmd(nc, in_maps, core_ids=list(range(NCORES)))
    return res.results[0]["out"].astype(np.float32)


def kernel(**inputs):
    return run(FULL, inputs)


# revision 16
# speedup vs baseline: 8.0619x; 1.6472x over previous
"""GCN+GIN graph encoder on 8 Trainium2 NeuronCores (Bass/Tile).

Math (reference):
  GCNConv:  h = relu(segsum_dst(norm_e * (x@W0)[src]) + b0),
            norm_e = dinv[src]*dinv[dst] over edges+self-loops,
            dinv = rsqrt(deg incl self-loop)
  GIN x2:   h = relu((h + segsum_dst(h[src])) @ Wg + bg)
  pool:     m = segment_mean(h, batch) -> relu(m@Wh1+bh1)@Wh2+bh2

Distribution: nodes (and their in-edges) sharded contiguously over 8 cores.
Per layer each core aggregates messages for its own dst nodes by gathering
rows of a replicated bf16 node-feature table (dma_gather on 4 SWDGE queues),
reducing edge tiles with one-hot selection matrices on the TensorEngine,
applying the layer linear transform W-stationary in feat-major, then
transposing back to node-major.  Tables are re-replicated between layers
with an AllGather; pooled partials are combined with an AllReduce and the
small MLP head is computed redundantly on every core.

Harness-cost driven design:
 * per-exec NEFF staging scales with STATIC instruction count -> each
   layer's per-block work runs in a single For_i hardware loop (every
   block padded to the same per-stream tile count), keeping the static
   program ~700 instructions instead of ~12k fully unrolled.
 * per-exec input shipping costs ~25 ms/MB and ~10 ms/array -> all
   per-core inputs are packed into ONE uint8 blob (bf16/int8/int16
   sections, bitcast-viewed on device), with edge streams in
   idx:int16 / doff:int8 / val:bf16 and features in bf16.

Aggregation identity per dst block b (128 dst nodes):
  aggT[f, d] = sum_e msg[e, f] * sel[e, d],  sel[e, d] = (doff[e] == d) * val[e]
computed as matmul(lhsT=msg_tile[128e, 128f], rhs=sel[128e, 128d]) accumulated
in PSUM over the block's edge tiles.  GCN folds dinv[src] into the table rows
(host-prescaled x) and dinv[dst] into val; GIN uses val=1 and a self-loop edge
supplies the "+h" term.  Pad edge slots carry doff=-1 -> zero contribution.
"""
import sys

sys.path.insert(0, '/opt/trn_rl_repo')

import numpy as np
import ml_dtypes

import concourse.bass as bass
import concourse.bacc as bacc
import concourse.mybir as mybir
import concourse.tile as tile
from concourse.bass import ds, ts
from concourse.bass_utils import run_bass_kernel_spmd
from concourse.masks import make_identity

F32 = mybir.dt.float32
BF16 = mybir.dt.bfloat16
I16 = mybir.dt.int16
I8 = mybir.dt.int8
U8 = mybir.dt.uint8
BF = ml_dtypes.bfloat16
P = 128
NCORES = 8
GMAX = 1024                 # max rows per dma_gather (single_packet limit)
NQ = 4                      # SWDGE queues
ALIGN = 512


class Cfg:
    def __init__(self, N, E, G, F, NHID, NOUT, NPN):
        self.N = N            # real nodes
        self.E = E            # edges (no self loops)
        self.G = G            # graphs
        self.F = F            # feature/hidden width (128)
        self.NHID = NHID
        self.NOUT = NOUT
        self.NPN = NPN        # real nodes per core
        assert NPN * NCORES >= N > NPN * (NCORES - 1)
        self.NPC = ((NPN + P - 1) // P) * P   # padded nodes per core
        self.NBLK = self.NPC // P
        self.NPAD = self.NPC * NCORES
        self.NHALF = self.NPAD // 2
        assert self.NHALF < 32768
        assert G == 2 * P


FULL = Cfg(N=50000, E=800000, G=256, F=128, NHID=256, NOUT=128, NPN=6250)


# ---------------------------------------------------------------- host prep
def preprocess(cfg, x, edge_index, batch, W0, b0, Wg1, bg1, Wg2, bg2,
               Wh1, bh1, Wh2, bh2):
    N, G, F = cfg.N, cfg.G, cfg.F
    NPN, NPC, NBLK, NHALF = cfg.NPN, cfg.NPC, cfg.NBLK, cfg.NHALF

    src = np.asarray(edge_index[0], dtype=np.int64)
    dst = np.asarray(edge_index[1], dtype=np.int64)
    batch = np.asarray(batch, dtype=np.int64)
    loop = np.arange(N, dtype=np.int64)
    s_all = np.concatenate([src, loop])
    d_all = np.concatenate([dst, loop])

    deg = np.bincount(d_all, minlength=N).astype(np.float64)
    dinv = (1.0 / np.sqrt(np.maximum(deg, 1.0))).astype(np.float32)

    def tabidx(n):
        c = n // NPN
        return c * NPC + (n - c * NPN)

    sidx = tabidx(s_all).astype(np.int64)
    c_e = d_all // NPN
    loc = d_all - c_e * NPN
    b_e = loc // P
    off_e = loc % P
    gblk = c_e * NBLK + b_e                      # global dst block id
    val_e = dinv[d_all].astype(np.float32)      # GCN dst scaling

    NGB = NCORES * NBLK
    streams = {}
    for name, mask in (("lo", sidx < NHALF), ("hi", sidx >= NHALF)):
        sg = gblk[mask]
        si = sidx[mask] - (0 if name == "lo" else NHALF)
        sof = off_e[mask]
        order = np.argsort(sg, kind="stable")
        sg, si, sof = sg[order], si[order], sof[order]
        cnt = np.bincount(sg, minlength=NGB)
        # uniform per-block tile count (same For_i body for every block/core)
        NT = int(np.ceil(cnt.max() / P))
        rows_blk = NT * P
        rows_core = NBLK * rows_blk
        starts = np.zeros(NGB, dtype=np.int64)
        starts[1:] = np.cumsum(cnt)[:-1]
        rank = np.arange(len(sg)) - np.repeat(starts, cnt)
        c_of = sg // NBLK
        b_of = sg % NBLK
        pos = c_of * rows_core + b_of * rows_blk + rank
        tot = NCORES * rows_core
        idx_arr = np.zeros(tot, dtype=np.int32)
        doff_arr = np.full(tot, -1, dtype=np.int8)
        idx_arr[pos] = si
        doff_arr[pos] = sof
        idx_arr = idx_arr.reshape(NCORES, NBLK, rows_blk)
        # gather chunk sizes within a block: full 1024s then the remainder
        chunks = [GMAX] * (rows_blk // GMAX)
        if rows_blk % GMAX:
            chunks.append(rows_blk % GMAX)
        cols_blk = rows_blk // 16
        wrapped = np.zeros((NCORES, 16, NBLK * cols_blk), dtype=np.int16)
        for b in range(NBLK):
            a = 0
            cc = b * cols_blk
            for sz in chunks:
                wrapped[:, :, cc:cc + sz // 16] = (
                    idx_arr[:, b, a:a + sz].reshape(NCORES, sz // 16, 16)
                    .swapaxes(1, 2))
                a += sz
                cc += sz // 16
        T = NBLK * NT
        doff2 = doff_arr.reshape(NCORES, T, P).transpose(0, 2, 1).copy()
        streams[name] = dict(NT=NT, T=T, chunks=chunks,
                             idx=wrapped, doff=doff2)

    # per-core node features: dinv[src]-prescaled, int8 with per-feature
    # scales (dequant s[f] and the dinv[dst] factor are folded into the
    # aggregation epilogue on device)
    x = np.asarray(x, dtype=np.float32)
    xt = x * dinv[:, None]
    s_feat = (np.abs(xt).max(axis=0) / 127.0).astype(np.float32)  # [F]
    xq_full = np.clip(np.round(xt / s_feat[None, :]), -127, 127).astype(np.int8)
    xs = np.zeros((NCORES, NPC, F), dtype=np.int8)
    dinv_rows = np.zeros((NCORES, 1, NPC), dtype=np.float32)
    for c in range(NCORES):
        lo_n = c * NPN
        hi_n = min(N, (c + 1) * NPN)
        n = hi_n - lo_n
        xs[c, :n] = xq_full[lo_n:hi_n]
        dinv_rows[c, 0, :n] = dinv[lo_n:hi_n]

    # pooling metadata: batch id per node, block-column-major, pad=-1
    cnt_g = np.bincount(batch, minlength=G).astype(np.float32)
    invc = (1.0 / np.maximum(cnt_g, 1.0)).astype(np.float32)
    bat = np.full((NCORES, P, NBLK), -1.0, dtype=BF)
    for c in range(NCORES):
        lo_n = c * NPN
        hi_n = min(N, (c + 1) * NPN)
        n = hi_n - lo_n
        colmaj = np.full(NPC, -1.0, dtype=np.float32)
        colmaj[:n] = batch[lo_n:hi_n].astype(np.float32)
        bat[c] = colmaj.reshape(NBLK, P).T.astype(BF)

    # weights bf16; wh2 packed [P, 2*NOUT] (chunk h at cols h*NOUT)
    wh2 = np.asarray(Wh2, np.float32)
    wh2pack = np.concatenate([wh2[0:P, :], wh2[P:2 * P, :]], axis=1).astype(BF)
    # f32 bias columns [P, 6]: b0, bg1, bg2, bh1_0, bh1_1, bh2
    bh1 = np.asarray(bh1, np.float32)
    bcols = np.stack([
        np.asarray(b0, np.float32), np.asarray(bg1, np.float32),
        np.asarray(bg2, np.float32), bh1[0:P].reshape(P), bh1[P:2 * P].reshape(P),
        np.asarray(bh2, np.float32)], axis=1).copy()

    common = [
        ("w0", np.asarray(W0, np.float32).astype(BF)),
        ("wg1", np.asarray(Wg1, np.float32).astype(BF)),
        ("wg2", np.asarray(Wg2, np.float32).astype(BF)),
        ("wh1", np.asarray(Wh1, np.float32).astype(BF)),
        ("wh2pack", wh2pack),
        ("bcols", bcols),
        ("invc", invc.reshape(1, G)),
        ("sfeat", s_feat.reshape(1, F)),
    ]

    # ---- pack per-core blobs
    sections = [
        ("xs", None), ("idxlo", None), ("idxhi", None),
        ("dofflo", None), ("doffhi", None), ("dinvrow", None),
        ("bat", None),
    ] + common
    percore = {
        "xs": xs,
        "idxlo": streams["lo"]["idx"], "idxhi": streams["hi"]["idx"],
        "dofflo": streams["lo"]["doff"], "doffhi": streams["hi"]["doff"],
        "dinvrow": dinv_rows,
        "bat": bat,
    }
    offs, off = {}, 0
    for nm, arr in sections:
        a = percore[nm][0] if arr is None else arr
        offs[nm] = off
        off += (a.nbytes + ALIGN - 1) // ALIGN * ALIGN
    BLOB = off
    blobs = np.zeros((NCORES, BLOB), np.uint8)
    for nm, arr in sections:
        for c in range(NCORES):
            a = percore[nm][c] if arr is None else arr
            raw = np.frombuffer(np.ascontiguousarray(a).tobytes(), np.uint8)
            blobs[c, offs[nm]:offs[nm] + raw.size] = raw

    in_maps = [dict(blob=blobs[c:c + 1]) for c in range(NCORES)]
    meta = dict(NTLO=streams["lo"]["NT"], CHLO=streams["lo"]["chunks"],
                NTHI=streams["hi"]["NT"], CHHI=streams["hi"]["chunks"],
                BLOB=BLOB, offs=offs)
    return in_maps, meta


# ---------------------------------------------------------------- program
def build_program(cfg, meta):
    NPC, NBLK, NPAD, NHALF = cfg.NPC, cfg.NBLK, cfg.NPAD, cfg.NHALF
    F, NHID, NOUT, G = cfg.F, cfg.NHID, cfg.NOUT, cfg.G
    NTLO, CHLO = meta["NTLO"], meta["CHLO"]
    NTHI, CHHI = meta["NTHI"], meta["CHHI"]
    TLO, THI = NBLK * NTLO, NBLK * NTHI
    CLO, CHI = TLO * 8, THI * 8          # idx cols (= rows/16) per core
    BLOB, offs = meta["BLOB"], meta["offs"]

    nc = bacc.Bacc(None, target_bir_lowering=False, debug=True,
                   num_devices=NCORES, num_swdge_queues=NQ)

    blob_d = nc.declare_dram_parameter("blob", [1, BLOB], U8, isOutput=False)
    out_d = nc.declare_dram_parameter("out", [G, NOUT], F32, isOutput=True)

    def view(nm, dt, rows, cols):
        esz = mybir.dt.size(dt)
        bc = blob_d.bitcast(dt)
        s = offs[nm] // esz
        return bc[0:1, s:s + rows * cols].rearrange("o (r c) -> (o r) c", c=cols)

    slice0 = nc.dram_tensor("slice0", [NPC, F], BF16)
    slice1 = nc.dram_tensor("slice1", [NPC, F], BF16)
    slice2 = nc.dram_tensor("slice2", [NPC, F], BF16)
    tab1 = nc.dram_tensor("tab1", [NPAD, F], BF16)
    tab2 = nc.dram_tensor("tab2", [NPAD, F], BF16)
    tab3 = nc.dram_tensor("tab3", [NPAD, F], BF16)
    pool_in = nc.dram_tensor("pool_in", [P, G], F32)
    pool_out = nc.dram_tensor("pool_out", [P, G], F32, addr_space="Shared")
    groups = [list(range(NCORES))]

    with tile.TileContext(nc) as tc:
        with (
            tc.tile_pool(name="const", bufs=1) as constp,
            tc.tile_pool(name="meta", bufs=1) as metap,
            tc.tile_pool(name="msg", bufs=2) as msgp,
            tc.tile_pool(name="sel", bufs=4) as selp,
            tc.tile_pool(name="work", bufs=4) as workp,
            tc.tile_pool(name="pagg", bufs=1, space="PSUM") as pagg,
            tc.tile_pool(name="phT", bufs=1, space="PSUM") as phT,
            tc.tile_pool(name="ptr", bufs=1, space="PSUM") as ptr,
            tc.tile_pool(name="ppool", bufs=1, space="PSUM") as ppool,
            tc.tile_pool(name="phead", bufs=1, space="PSUM") as phead,
        ):
            # ---- constants / metadata to SBUF
            ident = constp.tile([P, P], F32)
            make_identity(nc, ident[:])
            iota = constp.tile([P, P], BF16, tag="iota")
            nc.gpsimd.iota(iota[:], pattern=[[1, P]], base=0,
                           channel_multiplier=0,
                           allow_small_or_imprecise_dtypes=True)
            iotaG = constp.tile([P, G], BF16, tag="iotaG")
            nc.gpsimd.iota(iotaG[:], pattern=[[1, G]], base=0,
                           channel_multiplier=0,
                           allow_small_or_imprecise_dtypes=True)

            def load(nm, t_shape, dt=BF16, pool=metap):
                t = pool.tile(list(t_shape), dt, name=f"sb_{nm}", tag=f"sb_{nm}")
                nc.sync.dma_start(out=t[:], in_=view(nm, dt, *t_shape))
                return t

            # idx tables: ship [16, C], replicate to 128 partitions on device
            idxlo = metap.tile([P, CLO], I16, tag="idxlo")
            idxhi = metap.tile([P, CHI], I16, tag="idxhi")
            for k in range(8):
                nc.sync.dma_start(out=idxlo[16 * k:16 * (k + 1), :],
                                  in_=view("idxlo", I16, 16, CLO))
                nc.sync.dma_start(out=idxhi[16 * k:16 * (k + 1), :],
                                  in_=view("idxhi", I16, 16, CHI))
            dofflo8 = load("dofflo", [P, TLO], I8)
            doffhi8 = load("doffhi", [P, THI], I8)
            dofflo = metap.tile([P, TLO], F32, tag="dofflo_f")
            doffhi = metap.tile([P, THI], F32, tag="doffhi_f")
            nc.vector.tensor_copy(out=dofflo[:], in_=dofflo8[:])
            nc.vector.tensor_copy(out=doffhi[:], in_=doffhi8[:])
            w0 = load("w0", [F, F], pool=constp)
            wg1 = load("wg1", [F, F], pool=constp)
            wg2 = load("wg2", [F, F], pool=constp)
            wh1 = load("wh1", [F, NHID], pool=constp)
            wh2 = load("wh2pack", [P, 2 * NOUT], pool=constp)
            bcols = load("bcols", [P, 6], F32, pool=constp)
            batb = load("bat", [P, NBLK], pool=constp)
            bat = constp.tile([P, NBLK], F32, tag="bat_f")
            nc.vector.tensor_copy(out=bat[:], in_=batb[:])
            # invc broadcast [P, G] via rank-1 outer product ones x invc
            ones1 = constp.tile([1, P], F32, tag="ones1")
            nc.any.memset(ones1[:], 1.0)
            invc_row = load("invc", [1, G], F32, pool=constp)
            invb_ps = phead.tile([P, G], F32, space="PSUM", tag="ghead0")
            nc.tensor.matmul(out=invb_ps[:], lhsT=ones1[:], rhs=invc_row[:],
                             start=True, stop=True)
            invc_rep = constp.tile([P, G], F32, tag="invc_rep")
            nc.vector.tensor_copy(out=invc_rep[:], in_=invb_ps[:])
            # s_feat column [F, 1]: outer product s_row x [1]
            sfeat_row = load("sfeat", [1, F], F32, pool=constp)
            one11 = constp.tile([1, 1], F32, tag="one11")
            nc.any.memset(one11[:], 1.0)
            sc_ps = ptr.tile([P, P], F32, space="PSUM", tag="tr")
            nc.tensor.matmul(out=sc_ps[:, 0:1], lhsT=sfeat_row[:], rhs=one11[:],
                             start=True, stop=True)
            s_col = constp.tile([P, 1], F32, tag="s_col")
            nc.vector.tensor_copy(out=s_col[:], in_=sc_ps[:, 0:1])
            dinv_row = load("dinvrow", [1, NPC], F32, pool=constp)
            dinvrep = constp.tile([P, NPC], F32, tag="dinvrep")

            # stage xs: int8 -> bf16 into slice0, build dinvrep alongside
            with tc.For_i(0, NBLK, name="xstage") as i:
                xq = workp.tile([P, F], I8, tag="xq8")
                nc.sync.dma_start(
                    out=xq[:],
                    in_=view("xs", I8, NPC, F)[ts(i, P), :])
                xb = workp.tile([P, F], BF16, tag="xq_bf")
                nc.vector.tensor_copy(out=xb[:], in_=xq[:])
                nc.sync.dma_start(out=slice0[ts(i, P), :], in_=xb[:])
                dv_ps = ptr.tile([P, P], F32, space="PSUM", tag="tr")
                nc.tensor.matmul(out=dv_ps[:], lhsT=ones1[:],
                                 rhs=dinv_row[0:1, ts(i, P)],
                                 start=True, stop=True)
                nc.vector.tensor_copy(out=dinvrep[:, ts(i, P)], in_=dv_ps[:])
            nc.gpsimd.collective_compute(
                "AllGather", mybir.AluOpType.bypass, replica_groups=groups,
                ins=[slice0[:]], outs=[tab1[:]])

            pool_acc = constp.tile([P, G], F32, tag="pool_acc")

            def emit_layer(L, tab, W_sb, bias_col, dequant, out_slice):
                stream_info = [
                    ("lo", NTLO, CHLO, idxlo, dofflo, tab[0:NHALF, :]),
                    ("hi", NTHI, CHHI, idxhi, doffhi, tab[NHALF:NPAD, :]),
                ]
                with tc.For_i(0, NBLK, name=f"layer{L}") as i:
                    bufs = {}
                    qn = 0
                    for sname, NT, CH, idx_sb, _, tab_ap in stream_info:
                        buf = msgp.tile([P, NT * P], BF16, tag=f"buf{sname}")
                        bufs[sname] = buf
                        a = 0       # rows done within block
                        for sz in CH:
                            nc.gpsimd.dma_gather(
                                out_ap=buf[:, a:a + sz].rearrange(
                                    "p (c f) -> p c f", f=F),
                                in_ap=tab_ap,
                                idxs_ap=idx_sb[:, ds(i * (NT * 8) + a // 16,
                                                     sz // 16)],
                                num_idxs=sz, num_idxs_reg=sz,
                                elem_size=F, single_packet=True,
                                queue_num=qn % NQ)
                            qn += 1
                            a += sz
                    agg_ps = pagg.tile([P, F], F32, space="PSUM", tag="agg")
                    ntot = NTLO + NTHI
                    wi = 0
                    for sname, NT, CH, idx_sb, doff_sb, tab_ap in stream_info:
                        buf = bufs[sname]
                        for tt in range(NT):
                            sel = selp.tile([P, P], BF16, tag=f"sel{sname}{tt % 4}")
                            col = ds(i * NT + tt, 1)
                            nc.vector.tensor_scalar(
                                out=sel[:], in0=iota[:],
                                scalar1=doff_sb[:, col], scalar2=None,
                                op0=mybir.AluOpType.is_equal)
                            nc.tensor.matmul(
                                out=agg_ps[:],
                                lhsT=buf[:, tt * F:(tt + 1) * F],
                                rhs=sel[:], start=(wi == 0),
                                stop=(wi == ntot - 1))
                            wi += 1
                    aggT = workp.tile([P, F], BF16, tag="aggT")
                    if dequant:
                        # aggT[f, d] = agg_ps[f, d] * s_feat[f] * dinv[dst_d]
                        nc.vector.scalar_tensor_tensor(
                            out=aggT[:], in0=agg_ps[:], scalar=s_col[:, 0:1],
                            in1=dinvrep[:, ts(i, P)],
                            op0=mybir.AluOpType.mult,
                            op1=mybir.AluOpType.mult)
                    else:
                        nc.vector.tensor_copy(out=aggT[:], in_=agg_ps[:])
                    hT_ps = phT.tile([P, F], F32, space="PSUM", tag="hT")
                    nc.tensor.matmul(out=hT_ps[:], lhsT=W_sb[:], rhs=aggT[:],
                                     start=True, stop=True)
                    hT = workp.tile([P, F], F32, tag="hT_sb")
                    nc.scalar.activation(out=hT[:], in_=hT_ps[:],
                                         func=mybir.ActivationFunctionType.Relu,
                                         bias=bias_col)
                    h_ps = ptr.tile([P, F], F32, space="PSUM", tag="tr")
                    nc.tensor.transpose(out=h_ps[:], in_=hT[:], identity=ident[:])
                    h_sb = workp.tile([P, F], BF16, tag="h_sb")
                    nc.vector.tensor_copy(out=h_sb[:], in_=h_ps[:])
                    if out_slice is not None:
                        nc.sync.dma_start(out=out_slice[ts(i, P), :], in_=h_sb[:])
                    else:
                        # pool: one-hot [node -> graph] and accumulate [F, G]
                        selg = selp.tile([P, G], BF16, tag="selg")
                        nc.vector.tensor_scalar(
                            out=selg[:], in0=iotaG[:],
                            scalar1=bat[:, ds(i, 1)], scalar2=None,
                            op0=mybir.AluOpType.is_equal)
                        pmm = ppool.tile([P, G], F32, space="PSUM", tag="pmm")
                        nc.tensor.matmul(out=pmm[:], lhsT=h_sb[:], rhs=selg[:],
                                         start=True, stop=True)
                        nc.vector.tensor_add(out=pool_acc[:], in0=pool_acc[:],
                                             in1=pmm[:])

            emit_layer(0, tab1, w0, bcols[:, 0:1], True, slice1)
            nc.gpsimd.collective_compute(
                "AllGather", mybir.AluOpType.bypass, replica_groups=groups,
                ins=[slice1[:]], outs=[tab2[:]])
            emit_layer(1, tab2, wg1, bcols[:, 1:2], False, slice2)
            nc.gpsimd.collective_compute(
                "AllGather", mybir.AluOpType.bypass, replica_groups=groups,
                ins=[slice2[:]], outs=[tab3[:]])
            nc.any.memset(pool_acc[:], 0.0)
            emit_layer(2, tab3, wg2, bcols[:, 2:3], False, None)

            # ---- pooling: partial sums [F, G] -> AllReduce -> mean
            nc.sync.dma_start(out=pool_in[:], in_=pool_acc[:])
            nc.gpsimd.collective_compute(
                "AllReduce", mybir.AluOpType.add, replica_groups=groups,
                ins=[pool_in[:]], outs=[pool_out[:]])
            mT = workp.tile([P, G], F32, tag="mT")     # [F, G] mean, feat-major
            nc.sync.dma_start(out=mT[:], in_=pool_out[:])
            nc.vector.tensor_mul(out=mT[:], in0=mT[:], in1=invc_rep[:])
            mTb = workp.tile([P, G], BF16, tag="mTb")
            nc.vector.tensor_copy(out=mTb[:], in_=mT[:])

            # ---- head (redundant on every core), all graph-minor [*, G]
            g1T = []
            for h in range(NHID // P):
                g_ps = phead.tile([P, G], F32, space="PSUM", tag=f"ghead{h}")
                nc.tensor.matmul(out=g_ps[:], lhsT=wh1[:, h * P:(h + 1) * P],
                                 rhs=mTb[:], start=True, stop=True)
                gt = workp.tile([P, G], BF16, tag=f"g1T{h}")
                nc.scalar.activation(out=gt[:], in_=g_ps[:],
                                     func=mybir.ActivationFunctionType.Relu,
                                     bias=bcols[:, 3 + h:4 + h])
                g1T.append(gt)
            o_ps = phead.tile([P, G], F32, space="PSUM", tag="ohead")
            for h in range(NHID // P):
                nc.tensor.matmul(out=o_ps[:], lhsT=wh2[:, h * NOUT:(h + 1) * NOUT],
                                 rhs=g1T[h][:], start=(h == 0),
                                 stop=(h == NHID // P - 1))
            outT = workp.tile([P, G], F32, tag="outT")   # [NOUT, G]
            nc.vector.tensor_scalar(out=outT[:], in0=o_ps[:],
                                    scalar1=bcols[:, 5:6], scalar2=None,
                                    op0=mybir.AluOpType.add)
            for gc in range(G // P):
                tr_ps = ptr.tile([P, P], F32, space="PSUM", tag="tr")
                nc.tensor.transpose(out=tr_ps[:], in_=outT[:, gc * P:(gc + 1) * P],
                                    identity=ident[:])
                o_sb = workp.tile([P, NOUT], F32, tag=f"o{gc}")
                nc.vector.tensor_copy(out=o_sb[:], in_=tr_ps[:])
                nc.sync.dma_start(out=out_d[gc * P:(gc + 1) * P, :], in_=o_sb[:])

    nc.compile()
    return nc


_CACHE = {}


def run(cfg, inputs):
    in_maps, meta = preprocess(cfg, **inputs)
    key = (cfg.N, meta["NTLO"], meta["NTHI"], meta["BLOB"])
    if key not in _CACHE:
        _CACHE[key] = build_program(cfg, meta)
    nc = _CACHE[key]
    res = run_bass_kernel_spmd(nc, in_maps, core_ids=list(range(NCORES)))
    return res.results[0]["out"].astype(np.float32)


def kernel(**inputs):
    return run(FULL, inputs)


# revision 23
# speedup vs baseline: 8.3385x; 1.0343x over previous
"""GCN+GIN graph encoder on 8 Trainium2 NeuronCores (Bass/Tile).

Math (reference):
  GCNConv:  h = relu(segsum_dst(norm_e * (x@W0)[src]) + b0),
            norm_e = dinv[src]*dinv[dst] over edges+self-loops,
            dinv = rsqrt(deg incl self-loop)
  GIN x2:   h = relu((h + segsum_dst(h[src])) @ Wg + bg)
  pool:     m = segment_mean(h, batch) -> relu(m@Wh1+bh1)@Wh2+bh2

Distribution: nodes (and their in-edges) sharded contiguously over 8 cores.
Per layer each core aggregates messages for its own dst nodes by gathering
rows of a replicated bf16 node-feature table (dma_gather on 4 SWDGE queues),
reducing edge tiles with one-hot selection matrices on the TensorEngine,
applying the layer linear transform W-stationary in feat-major, then
transposing back to node-major.  Tables are re-replicated between layers
with an AllGather; pooled partials are combined with an AllReduce and the
small MLP head is computed redundantly on every core.

Harness-cost driven design:
 * per-exec NEFF staging scales with STATIC instruction count -> each
   layer's per-block work runs in a single For_i hardware loop (every
   block padded to the same per-stream tile count), keeping the static
   program ~700 instructions instead of ~12k fully unrolled.
 * per-exec input shipping costs ~25 ms/MB and ~10 ms/array -> all
   per-core inputs are packed into ONE uint8 blob (bf16/int8/int16
   sections, bitcast-viewed on device), with edge streams in
   idx:int16 / doff:int8 / val:bf16 and features in bf16.

Aggregation identity per dst block b (128 dst nodes):
  aggT[f, d] = sum_e msg[e, f] * sel[e, d],  sel[e, d] = (doff[e] == d) * val[e]
computed as matmul(lhsT=msg_tile[128e, 128f], rhs=sel[128e, 128d]) accumulated
in PSUM over the block's edge tiles.  GCN folds dinv[src] into the table rows
(host-prescaled x) and dinv[dst] into val; GIN uses val=1 and a self-loop edge
supplies the "+h" term.  Pad edge slots carry doff=-1 -> zero contribution.
"""
import sys

sys.path.insert(0, '/opt/trn_rl_repo')

import numpy as np
import ml_dtypes

import concourse.bass as bass
import concourse.bacc as bacc
import concourse.mybir as mybir
import concourse.tile as tile
from concourse.bass import ds, ts
from concourse.bass_utils import run_bass_kernel_spmd
from concourse.masks import make_identity

F32 = mybir.dt.float32
BF16 = mybir.dt.bfloat16
I16 = mybir.dt.int16
I8 = mybir.dt.int8
U8 = mybir.dt.uint8
BF = ml_dtypes.bfloat16
P = 128
NCORES = 8
GMAX = 1024                 # max rows per dma_gather (single_packet limit)
NQ = 4                      # SWDGE queues
ALIGN = 512


class Cfg:
    def __init__(self, N, E, G, F, NHID, NOUT, NPN):
        self.N = N            # real nodes
        self.E = E            # edges (no self loops)
        self.G = G            # graphs
        self.F = F            # feature/hidden width (128)
        self.NHID = NHID
        self.NOUT = NOUT
        self.NPN = NPN        # real nodes per core
        assert NPN * NCORES >= N > NPN * (NCORES - 1)
        self.NPC = ((NPN + P - 1) // P) * P   # padded nodes per core
        self.NBLK = self.NPC // P
        self.NPAD = self.NPC * NCORES
        self.NHALF = self.NPAD // 2
        assert self.NHALF < 32768
        assert G == 2 * P


FULL = Cfg(N=50000, E=800000, G=256, F=128, NHID=256, NOUT=128, NPN=6250)


# ---------------------------------------------------------------- host prep
def preprocess(cfg, x, edge_index, batch, W0, b0, Wg1, bg1, Wg2, bg2,
               Wh1, bh1, Wh2, bh2):
    N, G, F = cfg.N, cfg.G, cfg.F
    NPN, NPC, NBLK, NHALF = cfg.NPN, cfg.NPC, cfg.NBLK, cfg.NHALF

    src = np.asarray(edge_index[0], dtype=np.int64)
    dst = np.asarray(edge_index[1], dtype=np.int64)
    batch = np.asarray(batch, dtype=np.int64)
    loop = np.arange(N, dtype=np.int64)
    s_all = np.concatenate([src, loop])
    d_all = np.concatenate([dst, loop])

    deg = np.bincount(d_all, minlength=N).astype(np.float64)
    dinv = (1.0 / np.sqrt(np.maximum(deg, 1.0))).astype(np.float32)

    def tabidx(n):
        c = n // NPN
        return c * NPC + (n - c * NPN)

    sidx = tabidx(s_all).astype(np.int64)
    c_e = d_all // NPN
    loc = d_all - c_e * NPN
    b_e = loc // P
    off_e = loc % P
    gblk = c_e * NBLK + b_e                      # global dst block id
    val_e = dinv[d_all].astype(np.float32)      # GCN dst scaling

    NGB = NCORES * NBLK
    streams = {}
    for name, mask in (("lo", sidx < NHALF), ("hi", sidx >= NHALF)):
        sg = gblk[mask]
        si = sidx[mask] - (0 if name == "lo" else NHALF)
        sof = off_e[mask]
        order = np.argsort(sg, kind="stable")
        sg, si, sof = sg[order], si[order], sof[order]
        cnt = np.bincount(sg, minlength=NGB)
        # uniform per-block tile count (same For_i body for every block/core)
        NT = int(np.ceil(cnt.max() / P))
        rows_blk = NT * P
        rows_core = NBLK * rows_blk
        starts = np.zeros(NGB, dtype=np.int64)
        starts[1:] = np.cumsum(cnt)[:-1]
        rank = np.arange(len(sg)) - np.repeat(starts, cnt)
        c_of = sg // NBLK
        b_of = sg % NBLK
        pos = c_of * rows_core + b_of * rows_blk + rank
        tot = NCORES * rows_core
        idx_arr = np.zeros(tot, dtype=np.int32)
        doff_arr = np.full(tot, -1, dtype=np.int8)
        idx_arr[pos] = si
        doff_arr[pos] = sof
        idx_arr = idx_arr.reshape(NCORES, NBLK, rows_blk)
        # gather chunk sizes within a block: full 1024s then the remainder
        chunks = [GMAX] * (rows_blk // GMAX)
        if rows_blk % GMAX:
            chunks.append(rows_blk % GMAX)
        cols_blk = rows_blk // 16
        wrapped = np.zeros((NCORES, 16, NBLK * cols_blk), dtype=np.int16)
        for b in range(NBLK):
            a = 0
            cc = b * cols_blk
            for sz in chunks:
                wrapped[:, :, cc:cc + sz // 16] = (
                    idx_arr[:, b, a:a + sz].reshape(NCORES, sz // 16, 16)
                    .swapaxes(1, 2))
                a += sz
                cc += sz // 16
        T = NBLK * NT
        doff2 = doff_arr.reshape(NCORES, T, P).transpose(0, 2, 1).copy()
        streams[name] = dict(NT=NT, T=T, chunks=chunks,
                             idx=wrapped, doff=doff2)

    # per-core node features: dinv[src]-prescaled, int8 with per-feature
    # scales (dequant s[f] and the dinv[dst] factor are folded into the
    # aggregation epilogue on device)
    x = np.asarray(x, dtype=np.float32)
    xt = x * dinv[:, None]
    s_feat = (np.abs(xt).max(axis=0) / 127.0).astype(np.float32)  # [F]
    xq_full = np.clip(np.round(xt / s_feat[None, :]), -127, 127).astype(np.int8)
    xs = np.zeros((NCORES, NPC, F), dtype=np.int8)
    dinv_rows = np.zeros((NCORES, 1, NPC), dtype=np.float32)
    for c in range(NCORES):
        lo_n = c * NPN
        hi_n = min(N, (c + 1) * NPN)
        n = hi_n - lo_n
        xs[c, :n] = xq_full[lo_n:hi_n]
        dinv_rows[c, 0, :n] = dinv[lo_n:hi_n]

    # pooling metadata: batch id per node, block-column-major, pad=-1
    cnt_g = np.bincount(batch, minlength=G).astype(np.float32)
    invc = (1.0 / np.maximum(cnt_g, 1.0)).astype(np.float32)
    bat = np.full((NCORES, P, NBLK), -1.0, dtype=BF)
    for c in range(NCORES):
        lo_n = c * NPN
        hi_n = min(N, (c + 1) * NPN)
        n = hi_n - lo_n
        colmaj = np.full(NPC, -1.0, dtype=np.float32)
        colmaj[:n] = batch[lo_n:hi_n].astype(np.float32)
        bat[c] = colmaj.reshape(NBLK, P).T.astype(BF)

    # weights bf16; wh2 packed [P, 2*NOUT] (chunk h at cols h*NOUT)
    wh2 = np.asarray(Wh2, np.float32)
    wh2pack = np.concatenate([wh2[0:P, :], wh2[P:2 * P, :]], axis=1).astype(BF)
    # f32 bias columns [P, 6]: b0, bg1, bg2, bh1_0, bh1_1, bh2
    bh1 = np.asarray(bh1, np.float32)
    bcols = np.stack([
        np.asarray(b0, np.float32), np.asarray(bg1, np.float32),
        np.asarray(bg2, np.float32), bh1[0:P].reshape(P), bh1[P:2 * P].reshape(P),
        np.asarray(bh2, np.float32)], axis=1).copy()

    common = [
        ("w0", np.asarray(W0, np.float32).astype(BF)),
        ("wg1", np.asarray(Wg1, np.float32).astype(BF)),
        ("wg2", np.asarray(Wg2, np.float32).astype(BF)),
        ("wh1", np.asarray(Wh1, np.float32).astype(BF)),
        ("wh2pack", wh2pack),
        ("bcols", bcols),
        ("invc", invc.reshape(1, G)),
        ("sfeat", s_feat.reshape(1, F)),
    ]

    # ---- pack per-core blobs
    sections = [
        ("xs", None), ("idxlo", None), ("idxhi", None),
        ("dofflo", None), ("doffhi", None), ("dinvrow", None),
        ("bat", None),
    ] + common
    percore = {
        "xs": xs,
        "idxlo": streams["lo"]["idx"], "idxhi": streams["hi"]["idx"],
        "dofflo": streams["lo"]["doff"], "doffhi": streams["hi"]["doff"],
        "dinvrow": dinv_rows,
        "bat": bat,
    }
    offs, off = {}, 0
    for nm, arr in sections:
        a = percore[nm][0] if arr is None else arr
        offs[nm] = off
        off += (a.nbytes + ALIGN - 1) // ALIGN * ALIGN
    BLOB = off
    blobs = np.zeros((NCORES, BLOB), np.uint8)
    for nm, arr in sections:
        for c in range(NCORES):
            a = percore[nm][c] if arr is None else arr
            raw = np.frombuffer(np.ascontiguousarray(a).tobytes(), np.uint8)
            blobs[c, offs[nm]:offs[nm] + raw.size] = raw

    in_maps = [dict(blob=blobs[c:c + 1]) for c in range(NCORES)]
    meta = dict(NTLO=streams["lo"]["NT"], CHLO=streams["lo"]["chunks"],
                NTHI=streams["hi"]["NT"], CHHI=streams["hi"]["chunks"],
                BLOB=BLOB, offs=offs)
    return in_maps, meta


# ---------------------------------------------------------------- program
def build_program(cfg, meta):
    NPC, NBLK, NPAD, NHALF = cfg.NPC, cfg.NBLK, cfg.NPAD, cfg.NHALF
    F, NHID, NOUT, G = cfg.F, cfg.NHID, cfg.NOUT, cfg.G
    NTLO, CHLO = meta["NTLO"], meta["CHLO"]
    NTHI, CHHI = meta["NTHI"], meta["CHHI"]
    TLO, THI = NBLK * NTLO, NBLK * NTHI
    CLO, CHI = TLO * 8, THI * 8          # idx cols (= rows/16) per core
    BLOB, offs = meta["BLOB"], meta["offs"]

    nc = bacc.Bacc(None, target_bir_lowering=False, debug=True,
                   num_devices=NCORES, num_swdge_queues=NQ)

    GPC = G // NCORES        # graphs output per core (partition-id sliced)
    blob_d = nc.declare_dram_parameter("blob", [1, BLOB], U8, isOutput=False)
    out_d = nc.declare_dram_parameter("out", [GPC, NOUT], F32, isOutput=True)

    def view(nm, dt, rows, cols):
        esz = mybir.dt.size(dt)
        bc = blob_d.bitcast(dt)
        s = offs[nm] // esz
        return bc[0:1, s:s + rows * cols].rearrange("o (r c) -> (o r) c", c=cols)

    slice0 = nc.dram_tensor("slice0", [NPC, F], BF16)
    slice1 = nc.dram_tensor("slice1", [NPC, F], BF16)
    slice2 = nc.dram_tensor("slice2", [NPC, F], BF16)
    tab1 = nc.dram_tensor("tab1", [NPAD, F], BF16)
    tab2 = nc.dram_tensor("tab2", [NPAD, F], BF16)
    tab3 = nc.dram_tensor("tab3", [NPAD, F], BF16)
    pool_in = nc.dram_tensor("pool_in", [P, G], F32)
    pool_out = nc.dram_tensor("pool_out", [P, G], F32, addr_space="Shared")
    groups = [list(range(NCORES))]

    with tile.TileContext(nc) as tc:
        with (
            tc.tile_pool(name="const", bufs=1) as constp,
            tc.tile_pool(name="meta", bufs=1) as metap,
            tc.tile_pool(name="msg", bufs=2) as msgp,
            tc.tile_pool(name="sel", bufs=4) as selp,
            tc.tile_pool(name="work", bufs=4) as workp,
            tc.tile_pool(name="pagg", bufs=1, space="PSUM") as pagg,
            tc.tile_pool(name="phT", bufs=1, space="PSUM") as phT,
            tc.tile_pool(name="ptr", bufs=1, space="PSUM") as ptr,
            tc.tile_pool(name="ppool", bufs=1, space="PSUM") as ppool,
            tc.tile_pool(name="phead", bufs=1, space="PSUM") as phead,
        ):
            # ---- constants / metadata to SBUF
            ident = constp.tile([P, P], F32)
            make_identity(nc, ident[:])
            iota = constp.tile([P, P], F32, tag="iota")
            nc.gpsimd.iota(iota[:], pattern=[[1, P]], base=0,
                           channel_multiplier=0,
                           allow_small_or_imprecise_dtypes=True)
            iota4 = constp.tile([P, 4 * P], F32, tag="iota4")
            for k in range(4):
                nc.vector.tensor_copy(out=iota4[:, k * P:(k + 1) * P],
                                      in_=iota[:])
            iotaG = constp.tile([P, G], BF16, tag="iotaG")
            nc.gpsimd.iota(iotaG[:], pattern=[[1, G]], base=0,
                           channel_multiplier=0,
                           allow_small_or_imprecise_dtypes=True)

            def load(nm, t_shape, dt=BF16, pool=metap):
                t = pool.tile(list(t_shape), dt, name=f"sb_{nm}", tag=f"sb_{nm}")
                nc.sync.dma_start(out=t[:], in_=view(nm, dt, *t_shape))
                return t

            # idx tables: ship [16, C], replicate to 128 partitions on device
            idxlo = metap.tile([P, CLO], I16, tag="idxlo")
            idxhi = metap.tile([P, CHI], I16, tag="idxhi")
            for k in range(8):
                nc.sync.dma_start(out=idxlo[16 * k:16 * (k + 1), :],
                                  in_=view("idxlo", I16, 16, CLO))
                nc.sync.dma_start(out=idxhi[16 * k:16 * (k + 1), :],
                                  in_=view("idxhi", I16, 16, CHI))
            dofflo8 = load("dofflo", [P, TLO], I8)
            doffhi8 = load("doffhi", [P, THI], I8)
            dofflo = metap.tile([P, TLO], F32, tag="dofflo_f")
            doffhi = metap.tile([P, THI], F32, tag="doffhi_f")
            nc.vector.tensor_copy(out=dofflo[:], in_=dofflo8[:])
            nc.vector.tensor_copy(out=doffhi[:], in_=doffhi8[:])
            w0 = load("w0", [F, F], pool=constp)
            wg1 = load("wg1", [F, F], pool=constp)
            wg2 = load("wg2", [F, F], pool=constp)
            wh1 = load("wh1", [F, NHID], pool=constp)
            wh2 = load("wh2pack", [P, 2 * NOUT], pool=constp)
            bcols = load("bcols", [P, 6], F32, pool=constp)
            batb = load("bat", [P, NBLK], pool=constp)
            bat = constp.tile([P, NBLK], F32, tag="bat_f")
            nc.vector.tensor_copy(out=bat[:], in_=batb[:])
            # invc broadcast [P, G] via rank-1 outer product ones x invc
            ones1 = constp.tile([1, P], F32, tag="ones1")
            nc.any.memset(ones1[:], 1.0)
            invc_row = load("invc", [1, G], F32, pool=constp)
            invb_ps = phead.tile([P, G], F32, space="PSUM", tag="ghead0")
            nc.tensor.matmul(out=invb_ps[:], lhsT=ones1[:], rhs=invc_row[:],
                             start=True, stop=True)
            invc_rep = constp.tile([P, G], F32, tag="invc_rep")
            nc.vector.tensor_copy(out=invc_rep[:], in_=invb_ps[:])
            # s_feat column [F, 1]: outer product s_row x [1]
            sfeat_row = load("sfeat", [1, F], F32, pool=constp)
            one11 = constp.tile([1, 1], F32, tag="one11")
            nc.any.memset(one11[:], 1.0)
            sc_ps = ptr.tile([P, P], F32, space="PSUM", tag="tr")
            nc.tensor.matmul(out=sc_ps[:, 0:1], lhsT=sfeat_row[:], rhs=one11[:],
                             start=True, stop=True)
            s_col = constp.tile([P, 1], F32, tag="s_col")
            nc.vector.tensor_copy(out=s_col[:], in_=sc_ps[:, 0:1])
            dinv_row = load("dinvrow", [1, NPC], F32, pool=constp)
            dinvrep = constp.tile([P, NPC], F32, tag="dinvrep")

            # stage xs: int8 -> bf16 into slice0, build dinvrep alongside
            with tc.For_i(0, NBLK, name="xstage") as i:
                xq = workp.tile([P, F], I8, tag="xq8")
                nc.sync.dma_start(
                    out=xq[:],
                    in_=view("xs", I8, NPC, F)[ts(i, P), :])
                xb = workp.tile([P, F], BF16, tag="xq_bf")
                nc.vector.tensor_copy(out=xb[:], in_=xq[:])
                nc.sync.dma_start(out=slice0[ts(i, P), :], in_=xb[:])
                dv_ps = ptr.tile([P, P], F32, space="PSUM", tag="tr")
                nc.tensor.matmul(out=dv_ps[:], lhsT=ones1[:],
                                 rhs=dinv_row[0:1, ts(i, P)],
                                 start=True, stop=True)
                nc.vector.tensor_copy(out=dinvrep[:, ts(i, P)], in_=dv_ps[:])
            nc.gpsimd.collective_compute(
                "AllGather", mybir.AluOpType.bypass, replica_groups=groups,
                ins=[slice0[:]], outs=[tab1[:]])

            pool_acc = constp.tile([P, G], F32, tag="pool_acc")

            def emit_layer(L, tab, W_sb, bias_col, dequant, out_slice):
                stream_info = [
                    ("lo", NTLO, CHLO, idxlo, dofflo, tab[0:NHALF, :]),
                    ("hi", NTHI, CHHI, idxhi, doffhi, tab[NHALF:NPAD, :]),
                ]
                with tc.For_i(0, NBLK, name=f"layer{L}") as i:
                    bufs = {}
                    qn = 0
                    for sname, NT, CH, idx_sb, _, tab_ap in stream_info:
                        buf = msgp.tile([P, NT * P], BF16, tag=f"buf{sname}")
                        bufs[sname] = buf
                        a = 0       # rows done within block
                        for sz in CH:
                            nc.gpsimd.dma_gather(
                                out_ap=buf[:, a:a + sz].rearrange(
                                    "p (c f) -> p c f", f=F),
                                in_ap=tab_ap,
                                idxs_ap=idx_sb[:, ds(i * (NT * 8) + a // 16,
                                                     sz // 16)],
                                num_idxs=sz, num_idxs_reg=sz,
                                elem_size=F, single_packet=True,
                                queue_num=qn % NQ)
                            qn += 1
                            a += sz
                    agg_ps = pagg.tile([P, F], F32, space="PSUM", tag="agg")
                    ntot = NTLO + NTHI
                    wi = 0
                    for sname, NT, CH, idx_sb, doff_sb, tab_ap in stream_info:
                        buf = bufs[sname]
                        # one-hot selections built 4 tiles per DVE op:
                        # sel4[p, a, d] = (doff[p, i*NT+4g+a] == iota[d])
                        for g in range((NT + 3) // 4):
                            k0 = 4 * g
                            gsz = min(4, NT - k0)
                            sel4 = selp.tile([P, gsz * P], BF16,
                                             tag=f"sel{sname}{g}")
                            nc.vector.tensor_tensor(
                                out=sel4[:].rearrange("p (a b) -> p a b", b=P),
                                in0=doff_sb[:, ds(i * NT + k0, gsz)]
                                    .to_broadcast([P, gsz, P]),
                                in1=iota4[:, 0:gsz * P]
                                    .rearrange("p (a b) -> p a b", b=P),
                                op=mybir.AluOpType.is_equal)
                            for tt in range(gsz):
                                nc.tensor.matmul(
                                    out=agg_ps[:],
                                    lhsT=buf[:, (k0 + tt) * F:(k0 + tt + 1) * F],
                                    rhs=sel4[:, tt * P:(tt + 1) * P],
                                    start=(wi == 0),
                                    stop=(wi == ntot - 1))
                                wi += 1
                    aggT = workp.tile([P, F], BF16, tag="aggT")
                    if dequant:
                        # aggT[f, d] = agg_ps[f, d] * s_feat[f] * dinv[dst_d]
                        nc.vector.scalar_tensor_tensor(
                            out=aggT[:], in0=agg_ps[:], scalar=s_col[:, 0:1],
                            in1=dinvrep[:, ts(i, P)],
                            op0=mybir.AluOpType.mult,
                            op1=mybir.AluOpType.mult)
                    else:
                        nc.vector.tensor_copy(out=aggT[:], in_=agg_ps[:])
                    hT_ps = phT.tile([P, F], F32, space="PSUM", tag="hT")
                    nc.tensor.matmul(out=hT_ps[:], lhsT=W_sb[:], rhs=aggT[:],
                                     start=True, stop=True)
                    hT = workp.tile([P, F], F32, tag="hT_sb")
                    nc.scalar.activation(out=hT[:], in_=hT_ps[:],
                                         func=mybir.ActivationFunctionType.Relu,
                                         bias=bias_col)
                    h_ps = ptr.tile([P, F], F32, space="PSUM", tag="tr")
                    nc.tensor.transpose(out=h_ps[:], in_=hT[:], identity=ident[:])
                    h_sb = workp.tile([P, F], BF16, tag="h_sb")
                    nc.vector.tensor_copy(out=h_sb[:], in_=h_ps[:])
                    if out_slice is not None:
                        nc.sync.dma_start(out=out_slice[ts(i, P), :], in_=h_sb[:])
                    else:
                        # pool: one-hot [node -> graph] and accumulate [F, G]
                        selg = selp.tile([P, G], BF16, tag="selg")
                        nc.vector.tensor_scalar(
                            out=selg[:], in0=iotaG[:],
                            scalar1=bat[:, ds(i, 1)], scalar2=None,
                            op0=mybir.AluOpType.is_equal)
                        pmm = ppool.tile([P, G], F32, space="PSUM", tag="pmm")
                        nc.tensor.matmul(out=pmm[:], lhsT=h_sb[:], rhs=selg[:],
                                         start=True, stop=True)
                        nc.vector.tensor_add(out=pool_acc[:], in0=pool_acc[:],
                                             in1=pmm[:])

            emit_layer(0, tab1, w0, bcols[:, 0:1], True, slice1)
            nc.gpsimd.collective_compute(
                "AllGather", mybir.AluOpType.bypass, replica_groups=groups,
                ins=[slice1[:]], outs=[tab2[:]])
            emit_layer(1, tab2, wg1, bcols[:, 1:2], False, slice2)
            nc.gpsimd.collective_compute(
                "AllGather", mybir.AluOpType.bypass, replica_groups=groups,
                ins=[slice2[:]], outs=[tab3[:]])
            nc.any.memset(pool_acc[:], 0.0)
            emit_layer(2, tab3, wg2, bcols[:, 2:3], False, None)

            # ---- pooling: partial sums [F, G] -> AllReduce -> mean
            nc.sync.dma_start(out=pool_in[:], in_=pool_acc[:])
            nc.gpsimd.collective_compute(
                "AllReduce", mybir.AluOpType.add, replica_groups=groups,
                ins=[pool_in[:]], outs=[pool_out[:]])
            mT = workp.tile([P, G], F32, tag="mT")     # [F, G] mean, feat-major
            nc.sync.dma_start(out=mT[:], in_=pool_out[:])
            nc.vector.tensor_mul(out=mT[:], in0=mT[:], in1=invc_rep[:])
            mTb = workp.tile([P, G], BF16, tag="mTb")
            nc.vector.tensor_copy(out=mTb[:], in_=mT[:])

            # ---- head (redundant on every core), all graph-minor [*, G]
            g1T = []
            for h in range(NHID // P):
                g_ps = phead.tile([P, G], F32, space="PSUM", tag=f"ghead{h}")
                nc.tensor.matmul(out=g_ps[:], lhsT=wh1[:, h * P:(h + 1) * P],
                                 rhs=mTb[:], start=True, stop=True)
                gt = workp.tile([P, G], BF16, tag=f"g1T{h}")
                nc.scalar.activation(out=gt[:], in_=g_ps[:],
                                     func=mybir.ActivationFunctionType.Relu,
                                     bias=bcols[:, 3 + h:4 + h])
                g1T.append(gt)
            o_ps = phead.tile([P, G], F32, space="PSUM", tag="ohead")
            for h in range(NHID // P):
                nc.tensor.matmul(out=o_ps[:], lhsT=wh2[:, h * NOUT:(h + 1) * NOUT],
                                 rhs=g1T[h][:], start=(h == 0),
                                 stop=(h == NHID // P - 1))
            outT = workp.tile([P, G], F32, tag="outT")   # [NOUT, G]
            nc.vector.tensor_scalar(out=outT[:], in0=o_ps[:],
                                    scalar1=bcols[:, 5:6], scalar2=None,
                                    op0=mybir.AluOpType.add)
            # each core emits only its own GPC graphs (reassembled on host)
            pid = nc.vector.partition_id()
            oslice = workp.tile([P, GPC], F32, tag="oslice")
            nc.vector.tensor_copy(out=oslice[:], in_=outT[:, ds(pid * GPC, GPC)])
            tr_ps = ptr.tile([GPC, P], F32, space="PSUM", tag="otr")
            nc.tensor.transpose(out=tr_ps[:], in_=oslice[:], identity=ident[:])
            o_sb = workp.tile([GPC, NOUT], F32, tag="o_out")
            nc.vector.tensor_copy(out=o_sb[:], in_=tr_ps[:])
            nc.sync.dma_start(out=out_d[:], in_=o_sb[:])

    nc.compile()
    return nc


_CACHE = {}


def run(cfg, inputs):
    in_maps, meta = preprocess(cfg, **inputs)
    key = (cfg.N, meta["NTLO"], meta["NTHI"], meta["BLOB"])
    if key not in _CACHE:
        _CACHE[key] = build_program(cfg, meta)
    nc = _CACHE[key]
    res = run_bass_kernel_spmd(nc, in_maps, core_ids=list(range(NCORES)))
    return np.concatenate(
        [np.asarray(res.results[c]["out"]) for c in range(NCORES)],
        axis=0).astype(np.float32)


def kernel(**inputs):
    return run(FULL, inputs)
